# revision 5
# baseline (speedup 1.0000x reference)
"""Trainium2 Bass kernel for nn_DQNNModel (12-qubit, 8-layer DQNN, B=2048).

Self-contained: host-side numpy builds all gate/moving matrices; the device
runs a 15-pass state-stationary matmul pipeline over the [256, 4096] complex
statevector per core (batch sharded 8 ways across NeuronCores).

Math design (verified against the jax reference in numpy):
 - wires 0..11, wire w <-> index bit (11-w). G1 = wires 0..6 (128), G2 =
   wires 7..11 (32). Custom orders: G1 bits (w0, w6, w1..w5); G2 bits
   (w11, w8, w9, w10, w7).
 - Per layer j<=6: CNOT(6,7) = H7.CZ67.H7 and CNOT(11,0) = H0.CZ110.H0.
   CZs are diagonal and fold into class-split (col-tiled) matmuls whose
   movings carry sign flips; each layer's G2 gates ride in the PREVIOUS
   G2 pass (after the conditional Z), layer 0's in the initial product
   state. Layer 7's whole CNOT ring folds into the readout sign tile.
 - Passes: P1_j (contract G1) j=0..7, P2_j (contract G2) j=0..6.
   Layouts: L1 [p=G1^(128), f=(b_hi 64, ri 2, b_lo 4, rest1^ 32)],
            L2 [p=(w7, b_lo, q4)(128), f=(b_hi 64, ri 2, G1^ 128)].
"""
import os
import numpy as np

NQ, NL, SEQ, B, DIM = 12, 8, 64, 2048, 4096
NCORES = 8
BLOC = B // NCORES          # 256
NBH = BLOC // 4             # 64 b_hi chunks (4 batches each)
GRP = 4                     # chunks per PSUM group / eviction instr
DTYPE_NAME = os.environ.get("QKERNEL_DTYPE", "float32r")

# ---------------------------------------------------------------- host math
H2C = (1.0 / np.sqrt(2.0)) * np.array([[1, 1], [1, -1]], dtype=np.complex128)
I2C = np.eye(2, dtype=np.complex128)
G1_WIRES = [0, 6, 1, 2, 3, 4, 5]
G2_WIRES = [11, 8, 9, 10, 7]
G1_NAT = [0, 1, 2, 3, 4, 5, 6]
G2_NAT = [7, 8, 9, 10, 11]


def _order_perm(custom_wires, nat_wires):
    n = len(custom_wires)
    perm = np.zeros(2 ** n, dtype=np.int64)
    for ci in range(2 ** n):
        bits = {w: (ci >> (n - 1 - pos)) & 1 for pos, w in enumerate(custom_wires)}
        ni = 0
        for pos, w in enumerate(nat_wires):
            ni |= bits[w] << (n - 1 - pos)
        perm[ci] = ni
    return perm


P1O = _order_perm(G1_WIRES, G1_NAT)
P2O = _order_perm(G2_WIRES, G2_NAT)


def _kron_list(mats):
    out = np.array([[1.0]], dtype=np.complex128)
    for m in mats:
        out = np.kron(out, m)
    return out


def _op_on(group_nat, wire, U):
    return _kron_list([U if w == wire else I2C for w in group_nat])


def _cnot_in(group_nat, c, t):
    n = len(group_nat)
    pos = {w: n - 1 - i for i, w in enumerate(group_nat)}
    dim = 2 ** n
    M = np.zeros((dim, dim), dtype=np.complex128)
    for k in range(dim):
        cb = (k >> pos[c]) & 1
        M[k ^ (cb << pos[t]), k] = 1.0
    return M


def _rot_matrices(qw_layer):
    out = []
    for i in range(NQ):
        a, bb, g = qw_layer[i, 0] * 0.5, qw_layer[i, 1] * 0.5, qw_layer[i, 2] * 0.5
        ca, sa = np.cos(a), np.sin(a)
        cb, sb = np.cos(bb), np.sin(bb)
        m00 = cb * ca + 1j * sb * sa
        m01 = -(sb * ca) - 1j * cb * sa
        m10 = sb * ca - 1j * cb * sa
        m11 = cb * ca - 1j * sb * sa
        ez = np.exp(-1j * g)
        out.append(np.array([[ez * m00, ez * m01],
                             [np.conj(ez) * m10, np.conj(ez) * m11]]))
    return out


def _relabel(M, perm):
    return M[np.ix_(perm, perm)]


def _build_layer_ops(q_weights):
    H0 = _op_on(G1_NAT, 0, H2C)
    H7 = _op_on(G2_NAT, 7, H2C)
    Ch = np.eye(128, dtype=np.complex128)
    for c in range(6):
        Ch = _cnot_in(G1_NAT, c, c + 1) @ Ch
    Cl = np.eye(32, dtype=np.complex128)
    for c in range(7, 11):
        Cl = _cnot_in(G2_NAT, c, c + 1) @ Cl
    A_list, Bpre_list, Bpost_list = [], [], []
    for j in range(NL):
        Us = _rot_matrices(q_weights[:, j, :])
        UG1 = _kron_list([Us[w] for w in G1_NAT])
        conj = (j <= NL - 2)
        A = Ch @ UG1 if conj else UG1
        if conj:
            A = H0 @ A
        if j >= 1:
            A = A @ H0
        A_list.append(_relabel(A, P1O))
        if conj:
            Us_n = _rot_matrices(q_weights[:, j + 1, :])
            UG2n = _kron_list([Us_n[w] for w in G2_NAT])
            Bpre = Cl @ H7
            Bpost = UG2n if j + 1 > NL - 2 else (H7 @ UG2n)
            Bpre_list.append(_relabel(Bpre, P2O))
            Bpost_list.append(_relabel(Bpost, P2O))
    return A_list, Bpre_list, Bpost_list


def _init_factors(x, q_weights):
    ang = (np.pi * 0.5) * x.astype(np.float64)
    c, s = np.cos(ang), np.sin(ang)
    Bsz = x.shape[0]
    u = np.ones((Bsz, 1), dtype=np.float64)
    for w in G1_WIRES:
        u = (u[:, :, None] * np.stack([c[:, w], s[:, w]], -1)[:, None, :]).reshape(Bsz, -1)
    Us0 = _rot_matrices(q_weights[:, 0, :])
    v = np.ones((Bsz, 1), dtype=np.complex128)
    for w in G2_WIRES:
        f = np.stack([c[:, w], s[:, w]], -1).astype(np.complex128)
        M = Us0[w]
        if w == 7:
            M = H2C @ M
        f = f @ M.T
        v = (v[:, :, None] * f[:, None, :]).reshape(Bsz, -1)
    return u, v


def _ring_perm_map():
    F = np.arange(DIM, dtype=np.int64)
    pairs = [(c, c + 1) for c in range(NQ - 1)] + [(NQ - 1, 0)]
    for c, t in pairs:
        pc, pt = NQ - 1 - c, NQ - 1 - t
        v = np.arange(DIM)
        F = F[v ^ (((v >> pc) & 1) << pt)]
    return F


def _sign_tile(W_out):
    k = np.arange(DIM)
    signs = (1.0 - 2.0 * ((k[None, :] >> (NQ - 1 - np.arange(NQ))[:, None]) & 1))
    s_nat = (W_out[0].astype(np.float64) @ signs)
    s_eff = s_nat[np.argsort(_ring_perm_map())]
    kmat = (P1O[:, None] << 5) | P2O[None, :]
    return s_eff[kmat]                                  # [128, 32] (p^, l^)


def _build_movings(q_weights):
    """Per pass: dict(kind, classes=[(tile_col, m_cols, mov_r, mov_i)])."""
    A_list, Bpre_list, Bpost_list = _build_layer_ops(q_weights.astype(np.float64))
    passes = []
    h_idx = np.arange(128)
    negmask_h = ((h_idx & 32) == 32)                    # Z6: negate w6'=1 cols
    l_idx = np.arange(32)
    z11 = np.where((l_idx & 16) == 16, -1.0, 1.0)       # Z11 (w11 = l^ MSB)
    kk = np.arange(128)
    blk, lk = (kk >> 4) & 3, (kk & 15) * 2 + (kk >> 6)  # L2 part -> (b_lo, l^)

    for j in range(NL):
        conj = (j <= NL - 2)
        A = A_list[j]
        Ar, Ai = A.real.astype(np.float32), A.imag.astype(np.float32)
        mov_r = np.concatenate([Ar.T, Ai.T], axis=1).astype(np.float32)
        mov_i = np.concatenate([-Ai.T, Ar.T], axis=1).astype(np.float32)
        if conj:
            neg = np.concatenate([negmask_h, negmask_h])
            mov_r_B = mov_r.copy(); mov_r_B[:, neg] *= -1.0
            mov_i_B = mov_i.copy(); mov_i_B[:, neg] *= -1.0
            classes = [(0, 64, mov_r, mov_i), (64, 64, mov_r_B, mov_i_B)]
        else:
            # split into w7-halves anyway (walrus: weights AP must coalesce
            # to one free dim); same movings for both halves
            classes = [(0, 64, mov_r, mov_i), (64, 64, mov_r, mov_i)]
        passes.append(dict(kind='P1', classes=classes))
        if conj:
            MA = Bpost_list[j] @ Bpre_list[j]
            MB = Bpost_list[j] @ (z11[:, None] * Bpre_list[j])
            cls = []
            for tile_col, Mc in ((0, MA), (64, MB)):
                Mr = Mc.real.astype(np.float32)
                Mi = Mc.imag.astype(np.float32)
                movr = np.zeros((128, 256), dtype=np.float32)
                movi = np.zeros((128, 256), dtype=np.float32)
                for k in range(128):
                    base = blk[k] * 32
                    movr[k, base:base + 32] = Mr[:, lk[k]]
                    movr[k, 128 + base:128 + base + 32] = Mi[:, lk[k]]
                    movi[k, base:base + 32] = -Mi[:, lk[k]]
                    movi[k, 128 + base:128 + base + 32] = Mr[:, lk[k]]
                cls.append((tile_col, 64, movr, movi))
            passes.append(dict(kind='P2', classes=cls))
    return passes


# ---------------------------------------------------------------- device
def _build_program(pass_meta, nmov, dtype_name):
    """pass_meta: list of dict(kind, cls_idx=[(tile_col, m_cols, ir, ii)])."""
    import concourse.bass as bass
    import concourse.tile as tile
    from concourse import bacc, mybir

    f32 = mybir.dt.float32
    mmdt = getattr(mybir.dt, dtype_name)

    nc = bacc.Bacc("TRN2", target_bir_lowering=False, debug=False)
    u7_d = nc.dram_tensor("u7", [128, BLOC], f32, kind="ExternalInput").ap()
    v5r_d = nc.dram_tensor("v5r", [1, BLOC * 32], f32, kind="ExternalInput").ap()
    v5i_d = nc.dram_tensor("v5i", [1, BLOC * 32], f32, kind="ExternalInput").ap()
    s2_d = nc.dram_tensor("s2", [128, 256], f32, kind="ExternalInput").ap()
    bsel_d = nc.dram_tensor("bsel", [128, 4], f32, kind="ExternalInput").ap()
    movs_d = nc.dram_tensor("movs", [nmov, 128, 256], f32, kind="ExternalInput").ap()
    out_d = nc.dram_tensor("out", [BLOC], f32, kind="ExternalOutput").ap()

    def mk_ap(base_ap, dims):
        return bass.AP(base_ap.tensor, base_ap.offset, [list(base_ap.ap[0])] + dims)

    with tile.TileContext(nc) as tc:
        with (
            tc.tile_pool(name="state", bufs=1) as state_pool,
            tc.tile_pool(name="const", bufs=1) as const_pool,
            tc.tile_pool(name="v5p", bufs=2) as v5_pool,
            tc.tile_pool(name="pinit", bufs=2, space="PSUM") as psum_init,
            tc.tile_pool(name="pacc", bufs=2, space="PSUM") as psum_acc,
            tc.tile_pool(name="prd", bufs=1, space="PSUM") as psum_rd,
            tc.tile_pool(name="small", bufs=1) as small_pool,
        ):
            L1 = state_pool.tile([128, NBH * 256], f32, tag="L1")
            L2 = state_pool.tile([128, NBH * 256], f32, tag="L2")
            movs = const_pool.tile([128, nmov * 256], f32, tag="movs")
            u7 = const_pool.tile([128, BLOC], f32, tag="u7")
            s2 = const_pool.tile([128, 256], f32, tag="s2")
            ones1 = const_pool.tile([1, 128], f32, tag="ones1")
            bsel = const_pool.tile([128, 4], f32, tag="bsel")

            nc.sync.dma_start(u7[:], u7_d[:])
            nc.sync.dma_start(s2[:], s2_d[:])
            nc.sync.dma_start(bsel[:], bsel_d[:])
            for m in range(nmov):
                nc.sync.dma_start(movs[:, m * 256:(m + 1) * 256], movs_d[m])
            nc.vector.memset(ones1[:], 1.0)

            # ---- init: L1 = u7 (x) v5 (both planes) -----------------------
            # L1 col of (b, l^) plane ri: (b>>2)*256 + ri*128 + (b&3)*32 + l^
            for plane, v5d in ((0, v5r_d), (1, v5i_d)):
                for g in range(16):          # 512 cols = 16 batches each
                    v5c = v5_pool.tile([1, 512], f32, tag="v5c")
                    nc.sync.dma_start(v5c[:], v5d[:, g * 512:(g + 1) * 512])
                    pb = psum_init.tile([128, 512], f32, tag="initb")
                    nc.tensor.matmul(pb[:], ones1[:], v5c[:], start=True, stop=True)
                    out_ap = mk_ap(L1[:, g * 4 * 256 + plane * 128:],
                                   [[256, 4], [32, 4], [1, 32]])
                    in0 = mk_ap(pb[:], [[128, 4], [32, 4], [1, 32]])
                    in1 = mk_ap(u7[:, g * 16:], [[4, 4], [1, 4], [0, 32]])
                    nc.vector.tensor_tensor(out_ap, in0, in1, mybir.AluOpType.mult)

            # ---- 15 passes ------------------------------------------------
            for ip, pinfo in enumerate(pass_meta):
                p1 = pinfo['kind'] == 'P1'
                src, dst = (L1, L2) if p1 else (L2, L1)
                for grp in range(NBH // GRP):
                    ps = psum_acc.tile([128, GRP * 256], f32, tag="acc")
                    for ci in range(GRP):
                        ch = grp * GRP + ci
                        base = ch * 256
                        for (tile_col, mcols, ir, ii) in pinfo['cls_idx']:
                            if p1:
                                if mcols == 64:
                                    w7off = 1 if tile_col else 0
                                    stat_r = mk_ap(src[:, base + w7off:],
                                                   [[32, 4], [2, 16]])
                                    stat_i = mk_ap(src[:, base + 128 + w7off:],
                                                   [[32, 4], [2, 16]])
                                else:
                                    stat_r = mk_ap(src[:, base:],
                                                   [[1, 2], [32, 4], [2, 16]])
                                    stat_i = mk_ap(src[:, base + 128:],
                                                   [[1, 2], [32, 4], [2, 16]])
                            else:
                                stat_r = src[:, base + tile_col:base + tile_col + mcols]
                                stat_i = src[:, base + 128 + tile_col:base + 128 + tile_col + mcols]
                            out_ps = ps[tile_col:tile_col + mcols, ci * 256:(ci + 1) * 256]
                            tp = (0, tile_col) if mcols == 64 else None
                            mv_r = movs[:, ir * 256:(ir + 1) * 256]
                            mv_i = movs[:, ii * 256:(ii + 1) * 256]
                            if dtype_name != "float32":
                                stat_r = stat_r.bitcast(mmdt)
                                stat_i = stat_i.bitcast(mmdt)
                                mv_r = mv_r.bitcast(mmdt)
                                mv_i = mv_i.bitcast(mmdt)
                            nc.tensor.matmul(out_ps, stat_r, mv_r,
                                             start=True, stop=False, tile_position=tp)
                            nc.tensor.matmul(out_ps, stat_i, mv_i,
                                             start=False, stop=True, tile_position=tp)
                    dst_ap = dst[:, grp * GRP * 256:(grp + 1) * GRP * 256]
                    if grp % 3 != 2:
                        nc.vector.tensor_copy(dst_ap, ps[:])
                    else:
                        nc.scalar.copy(dst_ap, ps[:])

            # ---- readout from L2 ------------------------------------------
            nc.vector.tensor_tensor(L2[:], L2[:], L2[:], mybir.AluOpType.mult)
            s2b = mk_ap(s2[:], [[0, NBH], [1, 256]])
            l2seg = mk_ap(L2[:], [[256, NBH], [1, 256]])
            nc.vector.tensor_tensor(l2seg, l2seg, s2b, mybir.AluOpType.mult)
            R1 = small_pool.tile([128, NBH], f32, tag="R1")
            nc.vector.tensor_reduce(R1[:], l2seg, axis=mybir.AxisListType.X,
                                    op=mybir.AluOpType.add)
            pr = psum_rd.tile([4, NBH], f32, tag="rd")
            nc.tensor.matmul(pr[:], bsel[:], R1[:], start=True, stop=True)
            res = small_pool.tile([4, NBH], f32, tag="res")
            nc.scalar.copy(res[:], pr[:])
            out_ap = bass.AP(out_d.tensor, out_d.offset, [[1, 4], [4, NBH]])
            nc.sync.dma_start(out_ap, res[:])

    nc.compile()
    return nc


_PROGRAM_CACHE = {}


def _prepare_host(q_weights, W_out):
    passes = _build_movings(q_weights)
    mov_blobs, pass_meta = [], []
    for pinfo in passes:
        cls_idx = []
        for (tile_col, mcols, movr, movi) in pinfo['classes']:
            cls_idx.append((tile_col, mcols, len(mov_blobs), len(mov_blobs) + 1))
            mov_blobs.append(movr)
            mov_blobs.append(movi)
        pass_meta.append(dict(kind=pinfo['kind'], cls_idx=cls_idx))
    movs_arr = np.stack(mov_blobs).astype(np.float32)

    s_tile = _sign_tile(W_out).astype(np.float32)       # [h^, l^]
    kk = np.arange(128)
    lk = (kk & 15) * 2 + (kk >> 6)
    s2 = np.zeros((128, 256), dtype=np.float32)
    for k in range(128):
        s2[k, 0:128] = s_tile[:, lk[k]]
        s2[k, 128:256] = s_tile[:, lk[k]]
    return pass_meta, movs_arr, s2


def kernel(inputs, W_in, q_weights, W_out, b_out):
    from concourse.bass_utils import run_bass_kernel_spmd

    inputs = np.asarray(inputs, dtype=np.float32)
    W_in = np.asarray(W_in, dtype=np.float32)
    q_weights = np.asarray(q_weights, dtype=np.float32)
    W_out = np.asarray(W_out, dtype=np.float32)
    b_out = np.asarray(b_out, dtype=np.float32)

    x = inputs.astype(np.float64) @ W_in.T.astype(np.float64)     # [B, 12]
    u7o, v5o = _init_factors(x, q_weights.astype(np.float64))
    pass_meta, movs_arr, s2 = _prepare_host(q_weights, W_out)

    key = (DTYPE_NAME, movs_arr.shape[0])
    if key not in _PROGRAM_CACHE:
        _PROGRAM_CACHE[key] = _build_program(pass_meta, movs_arr.shape[0],
                                             DTYPE_NAME)
    nc = _PROGRAM_CACHE[key]

    kkp = np.arange(128)
    bsel_np = np.zeros((128, 4), dtype=np.float32)
    bsel_np[kkp, (kkp >> 4) & 3] = 1.0
    in_maps = []
    for c in range(NCORES):
        sl = slice(c * BLOC, (c + 1) * BLOC)
        in_maps.append({
            "u7": u7o[sl].T.astype(np.float32).copy(),
            "v5r": v5o[sl].real.astype(np.float32).reshape(1, -1).copy(),
            "v5i": v5o[sl].imag.astype(np.float32).reshape(1, -1).copy(),
            "s2": s2,
            "bsel": bsel_np,
            "movs": movs_arr,
        })
    trace = bool(int(os.environ.get("QKERNEL_TRACE", "0")))
    res = run_bass_kernel_spmd(nc, in_maps, list(range(NCORES)), trace=trace)
    global _LAST_RESULTS
    _LAST_RESULTS = res
    out = np.concatenate([res.results[c]["out"] for c in range(NCORES)])
    return (out + b_out[0]).astype(np.float32)[:, None]


_LAST_RESULTS = None


# revision 7
# speedup vs baseline: 2.1413x; 2.1413x over previous
"""Trainium2 Bass kernel for nn_DQNNModel (12-qubit, 8-layer DQNN, B=2048).

Self-contained: host-side numpy builds all gate/moving matrices; the device
runs a 15-pass state-stationary matmul pipeline over the [256, 4096] complex
statevector per core (batch sharded 8 ways across NeuronCores).

Math design (verified against the jax reference in numpy):
 - wires 0..11, wire w <-> index bit (11-w). G1 = wires 0..6 (128), G2 =
   wires 7..11 (32). Custom orders: G1 bits (w0, w6, w1..w5); G2 bits
   (w11, w8, w9, w10, w7).
 - Per layer j<=6: CNOT(6,7) = H7.CZ67.H7 and CNOT(11,0) = H0.CZ110.H0.
   CZs are diagonal and fold into class-split (col-tiled) matmuls whose
   movings carry sign flips; each layer's G2 gates ride in the PREVIOUS
   G2 pass (after the conditional Z), layer 0's in the initial product
   state. Layer 7's whole CNOT ring folds into the readout sign tile.
 - Passes: P1_j (contract G1) j=0..7, P2_j (contract G2) j=0..6.
   Layouts: L1 [p=G1^(128), f=(b_hi 64, ri 2, b_lo 4, rest1^ 32)],
            L2 [p=(w7, b_lo, q4)(128), f=(b_hi 64, ri 2, G1^ 128)].
"""
import os
import numpy as np

NQ, NL, SEQ, B, DIM = 12, 8, 64, 2048, 4096
NCORES = 8
BLOC = B // NCORES          # 256
NBH = BLOC // 4             # 64 b_hi chunks (4 batches each)
GRP = 4                     # chunks per PSUM group / eviction instr
DTYPE_NAME = os.environ.get("QKERNEL_DTYPE", "float32r")

# ---------------------------------------------------------------- host math
H2C = (1.0 / np.sqrt(2.0)) * np.array([[1, 1], [1, -1]], dtype=np.complex128)
I2C = np.eye(2, dtype=np.complex128)
G1_WIRES = [0, 6, 1, 2, 3, 4, 5]
G2_WIRES = [11, 8, 9, 10, 7]
G1_NAT = [0, 1, 2, 3, 4, 5, 6]
G2_NAT = [7, 8, 9, 10, 11]


def _order_perm(custom_wires, nat_wires):
    n = len(custom_wires)
    perm = np.zeros(2 ** n, dtype=np.int64)
    for ci in range(2 ** n):
        bits = {w: (ci >> (n - 1 - pos)) & 1 for pos, w in enumerate(custom_wires)}
        ni = 0
        for pos, w in enumerate(nat_wires):
            ni |= bits[w] << (n - 1 - pos)
        perm[ci] = ni
    return perm


P1O = _order_perm(G1_WIRES, G1_NAT)
P2O = _order_perm(G2_WIRES, G2_NAT)


def _kron_list(mats):
    out = np.array([[1.0]], dtype=np.complex128)
    for m in mats:
        out = np.kron(out, m)
    return out


def _op_on(group_nat, wire, U):
    return _kron_list([U if w == wire else I2C for w in group_nat])


def _cnot_in(group_nat, c, t):
    n = len(group_nat)
    pos = {w: n - 1 - i for i, w in enumerate(group_nat)}
    dim = 2 ** n
    M = np.zeros((dim, dim), dtype=np.complex128)
    for k in range(dim):
        cb = (k >> pos[c]) & 1
        M[k ^ (cb << pos[t]), k] = 1.0
    return M


def _rot_matrices(qw_layer):
    out = []
    for i in range(NQ):
        a, bb, g = qw_layer[i, 0] * 0.5, qw_layer[i, 1] * 0.5, qw_layer[i, 2] * 0.5
        ca, sa = np.cos(a), np.sin(a)
        cb, sb = np.cos(bb), np.sin(bb)
        m00 = cb * ca + 1j * sb * sa
        m01 = -(sb * ca) - 1j * cb * sa
        m10 = sb * ca - 1j * cb * sa
        m11 = cb * ca - 1j * sb * sa
        ez = np.exp(-1j * g)
        out.append(np.array([[ez * m00, ez * m01],
                             [np.conj(ez) * m10, np.conj(ez) * m11]]))
    return out


def _relabel(M, perm):
    return M[np.ix_(perm, perm)]


def _build_layer_ops(q_weights):
    H0 = _op_on(G1_NAT, 0, H2C)
    H7 = _op_on(G2_NAT, 7, H2C)
    Ch = np.eye(128, dtype=np.complex128)
    for c in range(6):
        Ch = _cnot_in(G1_NAT, c, c + 1) @ Ch
    Cl = np.eye(32, dtype=np.complex128)
    for c in range(7, 11):
        Cl = _cnot_in(G2_NAT, c, c + 1) @ Cl
    A_list, Bpre_list, Bpost_list = [], [], []
    for j in range(NL):
        Us = _rot_matrices(q_weights[:, j, :])
        UG1 = _kron_list([Us[w] for w in G1_NAT])
        conj = (j <= NL - 2)
        A = Ch @ UG1 if conj else UG1
        if conj:
            A = H0 @ A
        if j >= 1:
            A = A @ H0
        A_list.append(_relabel(A, P1O))
        if conj:
            Us_n = _rot_matrices(q_weights[:, j + 1, :])
            UG2n = _kron_list([Us_n[w] for w in G2_NAT])
            Bpre = Cl @ H7
            Bpost = UG2n if j + 1 > NL - 2 else (H7 @ UG2n)
            Bpre_list.append(_relabel(Bpre, P2O))
            Bpost_list.append(_relabel(Bpost, P2O))
    return A_list, Bpre_list, Bpost_list


def _init_factors(x, q_weights):
    ang = (np.pi * 0.5) * x.astype(np.float64)
    c, s = np.cos(ang), np.sin(ang)
    Bsz = x.shape[0]
    u = np.ones((Bsz, 1), dtype=np.float64)
    for w in G1_WIRES:
        u = (u[:, :, None] * np.stack([c[:, w], s[:, w]], -1)[:, None, :]).reshape(Bsz, -1)
    Us0 = _rot_matrices(q_weights[:, 0, :])
    v = np.ones((Bsz, 1), dtype=np.complex128)
    for w in G2_WIRES:
        f = np.stack([c[:, w], s[:, w]], -1).astype(np.complex128)
        M = Us0[w]
        if w == 7:
            M = H2C @ M
        f = f @ M.T
        v = (v[:, :, None] * f[:, None, :]).reshape(Bsz, -1)
    return u, v


def _ring_perm_map():
    F = np.arange(DIM, dtype=np.int64)
    pairs = [(c, c + 1) for c in range(NQ - 1)] + [(NQ - 1, 0)]
    for c, t in pairs:
        pc, pt = NQ - 1 - c, NQ - 1 - t
        v = np.arange(DIM)
        F = F[v ^ (((v >> pc) & 1) << pt)]
    return F


def _sign_tile(W_out):
    k = np.arange(DIM)
    signs = (1.0 - 2.0 * ((k[None, :] >> (NQ - 1 - np.arange(NQ))[:, None]) & 1))
    s_nat = (W_out[0].astype(np.float64) @ signs)
    s_eff = s_nat[np.argsort(_ring_perm_map())]
    kmat = (P1O[:, None] << 5) | P2O[None, :]
    return s_eff[kmat]                                  # [128, 32] (p^, l^)


def _build_movings(q_weights):
    """Per pass: dict(kind, classes=[(tile_col, m_cols, mov_r, mov_i)])."""
    A_list, Bpre_list, Bpost_list = _build_layer_ops(q_weights.astype(np.float64))
    passes = []
    h_idx = np.arange(128)
    negmask_h = ((h_idx & 32) == 32)                    # Z6: negate w6'=1 cols
    l_idx = np.arange(32)
    z11 = np.where((l_idx & 16) == 16, -1.0, 1.0)       # Z11 (w11 = l^ MSB)
    kk = np.arange(128)
    blk, lk = (kk >> 4) & 3, (kk & 15) * 2 + (kk >> 6)  # L2 part -> (b_lo, l^)

    for j in range(NL):
        conj = (j <= NL - 2)
        A = A_list[j]
        Ar, Ai = A.real.astype(np.float32), A.imag.astype(np.float32)
        mov_r = np.concatenate([Ar.T, Ai.T], axis=1).astype(np.float32)
        mov_i = np.concatenate([-Ai.T, Ar.T], axis=1).astype(np.float32)
        if conj:
            neg = np.concatenate([negmask_h, negmask_h])
            mov_r_B = mov_r.copy(); mov_r_B[:, neg] *= -1.0
            mov_i_B = mov_i.copy(); mov_i_B[:, neg] *= -1.0
            classes = [(0, 64, mov_r, mov_i), (64, 64, mov_r_B, mov_i_B)]
        else:
            # split into w7-halves anyway (walrus: weights AP must coalesce
            # to one free dim); same movings for both halves
            classes = [(0, 64, mov_r, mov_i), (64, 64, mov_r, mov_i)]
        passes.append(dict(kind='P1', classes=classes))
        if conj:
            MA = Bpost_list[j] @ Bpre_list[j]
            MB = Bpost_list[j] @ (z11[:, None] * Bpre_list[j])
            cls = []
            for tile_col, Mc in ((0, MA), (64, MB)):
                Mr = Mc.real.astype(np.float32)
                Mi = Mc.imag.astype(np.float32)
                movr = np.zeros((128, 256), dtype=np.float32)
                movi = np.zeros((128, 256), dtype=np.float32)
                for k in range(128):
                    base = blk[k] * 32
                    movr[k, base:base + 32] = Mr[:, lk[k]]
                    movr[k, 128 + base:128 + base + 32] = Mi[:, lk[k]]
                    movi[k, base:base + 32] = -Mi[:, lk[k]]
                    movi[k, 128 + base:128 + base + 32] = Mr[:, lk[k]]
                cls.append((tile_col, 64, movr, movi))
            passes.append(dict(kind='P2', classes=cls))
    return passes


# ---------------------------------------------------------------- device
def _build_program(pass_meta, nmov, dtype_name):
    """pass_meta: list of dict(kind, cls_idx=[(tile_col, m_cols, ir, ii)])."""
    import concourse.bass as bass
    import concourse.tile as tile
    from concourse import bacc, mybir

    f32 = mybir.dt.float32
    mmdt = getattr(mybir.dt, dtype_name)

    nc = bacc.Bacc("TRN2", target_bir_lowering=False, debug=False)
    u7_d = nc.dram_tensor("u7", [128, BLOC], f32, kind="ExternalInput").ap()
    v5r_d = nc.dram_tensor("v5r", [1, BLOC * 32], f32, kind="ExternalInput").ap()
    v5i_d = nc.dram_tensor("v5i", [1, BLOC * 32], f32, kind="ExternalInput").ap()
    s2_d = nc.dram_tensor("s2", [128, 256], f32, kind="ExternalInput").ap()
    bsel_d = nc.dram_tensor("bsel", [128, 4], f32, kind="ExternalInput").ap()
    movs_d = nc.dram_tensor("movs", [nmov, 128, 256], mmdt, kind="ExternalInput").ap()
    out_d = nc.dram_tensor("out", [BLOC], f32, kind="ExternalOutput").ap()

    def mk_ap(base_ap, dims):
        return bass.AP(base_ap.tensor, base_ap.offset, [list(base_ap.ap[0])] + dims)

    with tile.TileContext(nc) as tc:
        with (
            tc.tile_pool(name="state", bufs=1) as state_pool,
            tc.tile_pool(name="const", bufs=1) as const_pool,
            tc.tile_pool(name="v5p", bufs=2) as v5_pool,
            tc.tile_pool(name="pinit", bufs=2, space="PSUM") as psum_init,
            tc.tile_pool(name="pacc", bufs=2, space="PSUM") as psum_acc,
            tc.tile_pool(name="prd", bufs=1, space="PSUM") as psum_rd,
            tc.tile_pool(name="small", bufs=1) as small_pool,
        ):
            L1 = state_pool.tile([128, NBH * 256], mmdt, tag="L1")
            L2 = state_pool.tile([128, NBH * 256], mmdt, tag="L2")
            movs = const_pool.tile([128, nmov * 256], mmdt, tag="movs")
            u7 = const_pool.tile([128, BLOC], f32, tag="u7")
            s2 = const_pool.tile([128, 256], f32, tag="s2")
            ones1 = const_pool.tile([1, 128], f32, tag="ones1")
            bsel = const_pool.tile([128, 4], f32, tag="bsel")

            nc.sync.dma_start(u7[:], u7_d[:])
            nc.sync.dma_start(s2[:], s2_d[:])
            nc.sync.dma_start(bsel[:], bsel_d[:])
            for m in range(nmov):
                nc.sync.dma_start(movs[:, m * 256:(m + 1) * 256], movs_d[m])
            nc.vector.memset(ones1[:], 1.0)

            # ---- init: L1 = u7 (x) v5 (both planes) -----------------------
            # L1 col of (b, l^) plane ri: (b>>2)*256 + ri*128 + (b&3)*32 + l^
            for plane, v5d in ((0, v5r_d), (1, v5i_d)):
                for g in range(16):          # 512 cols = 16 batches each
                    v5c = v5_pool.tile([1, 512], f32, tag="v5c")
                    nc.sync.dma_start(v5c[:], v5d[:, g * 512:(g + 1) * 512])
                    pb = psum_init.tile([128, 512], f32, tag="initb")
                    nc.tensor.matmul(pb[:], ones1[:], v5c[:], start=True, stop=True)
                    out_ap = mk_ap(L1[:, g * 4 * 256 + plane * 128:],
                                   [[256, 4], [32, 4], [1, 32]])
                    in0 = mk_ap(pb[:], [[128, 4], [32, 4], [1, 32]])
                    in1 = mk_ap(u7[:, g * 16:], [[4, 4], [1, 4], [0, 32]])
                    nc.vector.tensor_tensor(out_ap, in0, in1, mybir.AluOpType.mult)

            # ---- 15 passes ------------------------------------------------
            for ip, pinfo in enumerate(pass_meta):
                p1 = pinfo['kind'] == 'P1'
                src, dst = (L1, L2) if p1 else (L2, L1)
                for grp in range(NBH // GRP):
                    ps = psum_acc.tile([128, GRP * 256], f32, tag="acc")
                    for ci in range(GRP):
                        ch = grp * GRP + ci
                        base = ch * 256
                        for (tile_col, mcols, ir, ii) in pinfo['cls_idx']:
                            if p1:
                                if mcols == 64:
                                    w7off = 1 if tile_col else 0
                                    stat_r = mk_ap(src[:, base + w7off:],
                                                   [[32, 4], [2, 16]])
                                    stat_i = mk_ap(src[:, base + 128 + w7off:],
                                                   [[32, 4], [2, 16]])
                                else:
                                    stat_r = mk_ap(src[:, base:],
                                                   [[1, 2], [32, 4], [2, 16]])
                                    stat_i = mk_ap(src[:, base + 128:],
                                                   [[1, 2], [32, 4], [2, 16]])
                            else:
                                stat_r = src[:, base + tile_col:base + tile_col + mcols]
                                stat_i = src[:, base + 128 + tile_col:base + 128 + tile_col + mcols]
                            out_ps = ps[tile_col:tile_col + mcols, ci * 256:(ci + 1) * 256]
                            tp = (0, tile_col) if mcols == 64 else None
                            mv_r = movs[:, ir * 256:(ir + 1) * 256]
                            mv_i = movs[:, ii * 256:(ii + 1) * 256]
                            nc.tensor.matmul(out_ps, stat_r, mv_r,
                                             start=True, stop=False, tile_position=tp)
                            nc.tensor.matmul(out_ps, stat_i, mv_i,
                                             start=False, stop=True, tile_position=tp)
                    dst_ap = dst[:, grp * GRP * 256:(grp + 1) * GRP * 256]
                    if grp % 3 != 2:
                        nc.vector.tensor_copy(dst_ap, ps[:])
                    else:
                        nc.scalar.copy(dst_ap, ps[:])

            # ---- readout from L2 ------------------------------------------
            if dtype_name == "float32":
                SQ = L2                      # square in place
            else:
                SQ = state_pool.tile([128, NBH * 256], f32, tag="SQ")
            nc.vector.tensor_tensor(SQ[:], L2[:], L2[:], mybir.AluOpType.mult)
            s2b = mk_ap(s2[:], [[0, NBH], [1, 256]])
            sqseg = mk_ap(SQ[:], [[256, NBH], [1, 256]])
            nc.vector.tensor_tensor(sqseg, sqseg, s2b, mybir.AluOpType.mult)
            R1 = small_pool.tile([128, NBH], f32, tag="R1")
            nc.vector.tensor_reduce(R1[:], sqseg, axis=mybir.AxisListType.X,
                                    op=mybir.AluOpType.add)
            pr = psum_rd.tile([4, NBH], f32, tag="rd")
            nc.tensor.matmul(pr[:], bsel[:], R1[:], start=True, stop=True)
            res = small_pool.tile([4, NBH], f32, tag="res")
            nc.scalar.copy(res[:], pr[:])
            out_ap = bass.AP(out_d.tensor, out_d.offset, [[1, 4], [4, NBH]])
            nc.sync.dma_start(out_ap, res[:])

    nc.compile()
    return nc


_PROGRAM_CACHE = {}


def _prepare_host(q_weights, W_out):
    passes = _build_movings(q_weights)
    mov_blobs, pass_meta = [], []
    for pinfo in passes:
        cls_idx = []
        for (tile_col, mcols, movr, movi) in pinfo['classes']:
            cls_idx.append((tile_col, mcols, len(mov_blobs), len(mov_blobs) + 1))
            mov_blobs.append(movr)
            mov_blobs.append(movi)
        pass_meta.append(dict(kind=pinfo['kind'], cls_idx=cls_idx))
    movs_arr = np.stack(mov_blobs).astype(np.float32)

    s_tile = _sign_tile(W_out).astype(np.float32)       # [h^, l^]
    kk = np.arange(128)
    lk = (kk & 15) * 2 + (kk >> 6)
    s2 = np.zeros((128, 256), dtype=np.float32)
    for k in range(128):
        s2[k, 0:128] = s_tile[:, lk[k]]
        s2[k, 128:256] = s_tile[:, lk[k]]
    return pass_meta, movs_arr, s2


def kernel(inputs, W_in, q_weights, W_out, b_out):
    from concourse.bass_utils import run_bass_kernel_spmd

    inputs = np.asarray(inputs, dtype=np.float32)
    W_in = np.asarray(W_in, dtype=np.float32)
    q_weights = np.asarray(q_weights, dtype=np.float32)
    W_out = np.asarray(W_out, dtype=np.float32)
    b_out = np.asarray(b_out, dtype=np.float32)

    x = inputs.astype(np.float64) @ W_in.T.astype(np.float64)     # [B, 12]
    u7o, v5o = _init_factors(x, q_weights.astype(np.float64))
    pass_meta, movs_arr, s2 = _prepare_host(q_weights, W_out)

    key = (DTYPE_NAME, movs_arr.shape[0])
    if key not in _PROGRAM_CACHE:
        _PROGRAM_CACHE[key] = _build_program(pass_meta, movs_arr.shape[0],
                                             DTYPE_NAME)
    nc = _PROGRAM_CACHE[key]

    movs_cast = movs_arr
    if DTYPE_NAME == "float16":
        movs_cast = movs_arr.astype(np.float16)
    elif DTYPE_NAME == "bfloat16":
        import ml_dtypes
        movs_cast = movs_arr.astype(ml_dtypes.bfloat16)
    kkp = np.arange(128)
    bsel_np = np.zeros((128, 4), dtype=np.float32)
    bsel_np[kkp, (kkp >> 4) & 3] = 1.0
    in_maps = []
    for c in range(NCORES):
        sl = slice(c * BLOC, (c + 1) * BLOC)
        in_maps.append({
            "u7": u7o[sl].T.astype(np.float32).copy(),
            "v5r": v5o[sl].real.astype(np.float32).reshape(1, -1).copy(),
            "v5i": v5o[sl].imag.astype(np.float32).reshape(1, -1).copy(),
            "s2": s2,
            "bsel": bsel_np,
            "movs": movs_cast,
        })
    trace = bool(int(os.environ.get("QKERNEL_TRACE", "0")))
    res = run_bass_kernel_spmd(nc, in_maps, list(range(NCORES)), trace=trace)
    global _LAST_RESULTS
    _LAST_RESULTS = res
    out = np.concatenate([res.results[c]["out"] for c in range(NCORES)])
    return (out + b_out[0]).astype(np.float32)[:, None]


_LAST_RESULTS = None


# revision 9
# speedup vs baseline: 2.5512x; 1.1914x over previous
"""Trainium2 Bass kernel for nn_DQNNModel (12-qubit, 8-layer DQNN, B=2048).

Self-contained: host-side numpy builds all gate/moving matrices; the device
runs a 15-pass state-stationary matmul pipeline over the [256, 4096] complex
statevector per core (batch sharded 8 ways across NeuronCores).

Math design (verified against the jax reference in numpy):
 - wires 0..11, wire w <-> index bit (11-w). G1 = wires 0..6 (128), G2 =
   wires 7..11 (32). Custom orders: G1 bits (w0, w6, w1..w5); G2 bits
   (w11, w8, w9, w10, w7).
 - Per layer j<=6: CNOT(6,7) = H7.CZ67.H7 and CNOT(11,0) = H0.CZ110.H0.
   CZs are diagonal and fold into class-split (col-tiled) matmuls whose
   movings carry sign flips; each layer's G2 gates ride in the PREVIOUS
   G2 pass (after the conditional Z), layer 0's in the initial product
   state. Layer 7's whole CNOT ring folds into the readout sign tile.
 - Passes: P1_j (contract G1) j=0..7, P2_j (contract G2) j=0..6.
   Layouts: L1 [p=G1^(128), f=(b_hi 64, ri 2, b_lo 4, rest1^ 32)],
            L2 [p=(w7, b_lo, q4)(128), f=(b_hi 64, ri 2, G1^ 128)].
"""
import os
import numpy as np

NQ, NL, SEQ, B, DIM = 12, 8, 64, 2048, 4096
NCORES = 8
BLOC = B // NCORES          # 256
NBH = BLOC // 4             # 64 b_hi chunks (4 batches each)
GRP = 4                     # chunks per PSUM group / eviction instr
DTYPE_NAME = os.environ.get("QKERNEL_DTYPE", "float32r")

# ---------------------------------------------------------------- host math
H2C = (1.0 / np.sqrt(2.0)) * np.array([[1, 1], [1, -1]], dtype=np.complex128)
I2C = np.eye(2, dtype=np.complex128)
G1_WIRES = [0, 6, 1, 2, 3, 4, 5]
G2_WIRES = [11, 8, 9, 10, 7]
G1_NAT = [0, 1, 2, 3, 4, 5, 6]
G2_NAT = [7, 8, 9, 10, 11]


def _order_perm(custom_wires, nat_wires):
    n = len(custom_wires)
    perm = np.zeros(2 ** n, dtype=np.int64)
    for ci in range(2 ** n):
        bits = {w: (ci >> (n - 1 - pos)) & 1 for pos, w in enumerate(custom_wires)}
        ni = 0
        for pos, w in enumerate(nat_wires):
            ni |= bits[w] << (n - 1 - pos)
        perm[ci] = ni
    return perm


P1O = _order_perm(G1_WIRES, G1_NAT)
P2O = _order_perm(G2_WIRES, G2_NAT)


def _kron_list(mats):
    out = np.array([[1.0]], dtype=np.complex128)
    for m in mats:
        out = np.kron(out, m)
    return out


def _op_on(group_nat, wire, U):
    return _kron_list([U if w == wire else I2C for w in group_nat])


def _cnot_in(group_nat, c, t):
    n = len(group_nat)
    pos = {w: n - 1 - i for i, w in enumerate(group_nat)}
    dim = 2 ** n
    M = np.zeros((dim, dim), dtype=np.complex128)
    for k in range(dim):
        cb = (k >> pos[c]) & 1
        M[k ^ (cb << pos[t]), k] = 1.0
    return M


def _rot_matrices(qw_layer):
    out = []
    for i in range(NQ):
        a, bb, g = qw_layer[i, 0] * 0.5, qw_layer[i, 1] * 0.5, qw_layer[i, 2] * 0.5
        ca, sa = np.cos(a), np.sin(a)
        cb, sb = np.cos(bb), np.sin(bb)
        m00 = cb * ca + 1j * sb * sa
        m01 = -(sb * ca) - 1j * cb * sa
        m10 = sb * ca - 1j * cb * sa
        m11 = cb * ca - 1j * sb * sa
        ez = np.exp(-1j * g)
        out.append(np.array([[ez * m00, ez * m01],
                             [np.conj(ez) * m10, np.conj(ez) * m11]]))
    return out


def _relabel(M, perm):
    return M[np.ix_(perm, perm)]


def _build_layer_ops(q_weights):
    H0 = _op_on(G1_NAT, 0, H2C)
    H7 = _op_on(G2_NAT, 7, H2C)
    Ch = np.eye(128, dtype=np.complex128)
    for c in range(6):
        Ch = _cnot_in(G1_NAT, c, c + 1) @ Ch
    Cl = np.eye(32, dtype=np.complex128)
    for c in range(7, 11):
        Cl = _cnot_in(G2_NAT, c, c + 1) @ Cl
    A_list, Bpre_list, Bpost_list = [], [], []
    for j in range(NL):
        Us = _rot_matrices(q_weights[:, j, :])
        UG1 = _kron_list([Us[w] for w in G1_NAT])
        conj = (j <= NL - 2)
        A = Ch @ UG1 if conj else UG1
        if conj:
            A = H0 @ A
        if j >= 1:
            A = A @ H0
        A_list.append(_relabel(A, P1O))
        if conj:
            Us_n = _rot_matrices(q_weights[:, j + 1, :])
            UG2n = _kron_list([Us_n[w] for w in G2_NAT])
            Bpre = Cl @ H7
            Bpost = UG2n if j + 1 > NL - 2 else (H7 @ UG2n)
            Bpre_list.append(_relabel(Bpre, P2O))
            Bpost_list.append(_relabel(Bpost, P2O))
    return A_list, Bpre_list, Bpost_list


def _init_factors(x, q_weights):
    ang = (np.pi * 0.5) * x.astype(np.float64)
    c, s = np.cos(ang), np.sin(ang)
    Bsz = x.shape[0]
    u = np.ones((Bsz, 1), dtype=np.float64)
    for w in G1_WIRES:
        u = (u[:, :, None] * np.stack([c[:, w], s[:, w]], -1)[:, None, :]).reshape(Bsz, -1)
    Us0 = _rot_matrices(q_weights[:, 0, :])
    v = np.ones((Bsz, 1), dtype=np.complex128)
    for w in G2_WIRES:
        f = np.stack([c[:, w], s[:, w]], -1).astype(np.complex128)
        M = Us0[w]
        if w == 7:
            M = H2C @ M
        f = f @ M.T
        v = (v[:, :, None] * f[:, None, :]).reshape(Bsz, -1)
    return u, v


def _ring_perm_map():
    F = np.arange(DIM, dtype=np.int64)
    pairs = [(c, c + 1) for c in range(NQ - 1)] + [(NQ - 1, 0)]
    for c, t in pairs:
        pc, pt = NQ - 1 - c, NQ - 1 - t
        v = np.arange(DIM)
        F = F[v ^ (((v >> pc) & 1) << pt)]
    return F


def _sign_tile(W_out):
    k = np.arange(DIM)
    signs = (1.0 - 2.0 * ((k[None, :] >> (NQ - 1 - np.arange(NQ))[:, None]) & 1))
    s_nat = (W_out[0].astype(np.float64) @ signs)
    s_eff = s_nat[np.argsort(_ring_perm_map())]
    kmat = (P1O[:, None] << 5) | P2O[None, :]
    return s_eff[kmat]                                  # [128, 32] (p^, l^)


def _build_movings(q_weights):
    """Per pass: dict(kind, classes=[(tile_col, m_cols, mov_r, mov_i)])."""
    A_list, Bpre_list, Bpost_list = _build_layer_ops(q_weights.astype(np.float64))
    passes = []
    h_idx = np.arange(128)
    negmask_h = ((h_idx & 32) == 32)                    # Z6: negate w6'=1 cols
    l_idx = np.arange(32)
    z11 = np.where((l_idx & 16) == 16, -1.0, 1.0)       # Z11 (w11 = l^ MSB)
    kk = np.arange(128)
    blk, lk = (kk >> 4) & 3, (kk & 15) * 2 + (kk >> 6)  # L2 part -> (b_lo, l^)

    for j in range(NL):
        conj = (j <= NL - 2)
        A = A_list[j]
        Ar, Ai = A.real.astype(np.float32), A.imag.astype(np.float32)
        mov_r = np.concatenate([Ar.T, Ai.T], axis=1).astype(np.float32)
        mov_i = np.concatenate([-Ai.T, Ar.T], axis=1).astype(np.float32)
        if conj:
            neg = np.concatenate([negmask_h, negmask_h])
            mov_r_B = mov_r.copy(); mov_r_B[:, neg] *= -1.0
            mov_i_B = mov_i.copy(); mov_i_B[:, neg] *= -1.0
            classes = [(0, 64, mov_r, mov_i), (64, 64, mov_r_B, mov_i_B)]
        else:
            # split into w7-halves anyway (walrus: weights AP must coalesce
            # to one free dim); same movings for both halves
            classes = [(0, 64, mov_r, mov_i), (64, 64, mov_r, mov_i)]
        passes.append(dict(kind='P1', classes=classes))
        if conj:
            MA = Bpost_list[j] @ Bpre_list[j]
            MB = Bpost_list[j] @ (z11[:, None] * Bpre_list[j])
            cls = []
            for tile_col, Mc in ((0, MA), (64, MB)):
                Mr = Mc.real.astype(np.float32)
                Mi = Mc.imag.astype(np.float32)
                movr = np.zeros((128, 256), dtype=np.float32)
                movi = np.zeros((128, 256), dtype=np.float32)
                for k in range(128):
                    base = blk[k] * 32
                    movr[k, base:base + 32] = Mr[:, lk[k]]
                    movr[k, 128 + base:128 + base + 32] = Mi[:, lk[k]]
                    movi[k, base:base + 32] = -Mi[:, lk[k]]
                    movi[k, 128 + base:128 + base + 32] = Mr[:, lk[k]]
                cls.append((tile_col, 64, movr, movi))
            passes.append(dict(kind='P2', classes=cls))
    return passes


# ---------------------------------------------------------------- device
def _build_program(pass_meta, nmov, dtype_name):
    """pass_meta: list of dict(kind, cls_idx=[(tile_col, m_cols, ir, ii)])."""
    import concourse.bass as bass
    import concourse.tile as tile
    from concourse import bacc, mybir

    f32 = mybir.dt.float32
    mmdt = getattr(mybir.dt, dtype_name)

    nc = bacc.Bacc("TRN2", target_bir_lowering=False, debug=False)
    u7_d = nc.dram_tensor("u7", [128, BLOC], f32, kind="ExternalInput").ap()
    v5r_d = nc.dram_tensor("v5r", [1, BLOC * 32], f32, kind="ExternalInput").ap()
    v5i_d = nc.dram_tensor("v5i", [1, BLOC * 32], f32, kind="ExternalInput").ap()
    s2_d = nc.dram_tensor("s2", [128, 256], f32, kind="ExternalInput").ap()
    bsel_d = nc.dram_tensor("bsel", [128, 4], f32, kind="ExternalInput").ap()
    movs_d = nc.dram_tensor("movs", [nmov, 128, 256], mmdt, kind="ExternalInput").ap()
    out_d = nc.dram_tensor("out", [BLOC], f32, kind="ExternalOutput").ap()

    def mk_ap(base_ap, dims):
        return bass.AP(base_ap.tensor, base_ap.offset, [list(base_ap.ap[0])] + dims)

    with tile.TileContext(nc) as tc:
        with (
            tc.tile_pool(name="state", bufs=1) as state_pool,
            tc.tile_pool(name="const", bufs=1) as const_pool,
            tc.tile_pool(name="v5p", bufs=2) as v5_pool,
            tc.tile_pool(name="pinit", bufs=1, space="PSUM") as psum_init,
            tc.tile_pool(name="pacc", bufs=3, space="PSUM") as psum_acc,
            tc.tile_pool(name="prd", bufs=1, space="PSUM") as psum_rd,
            tc.tile_pool(name="small", bufs=1) as small_pool,
        ):
            L1 = state_pool.tile([128, NBH * 256], mmdt, tag="L1")
            L2 = state_pool.tile([128, NBH * 256], mmdt, tag="L2")
            movs = const_pool.tile([128, nmov * 256], mmdt, tag="movs")
            u7 = const_pool.tile([128, BLOC], f32, tag="u7")
            s2 = const_pool.tile([128, 256], f32, tag="s2")
            ones1 = const_pool.tile([1, 128], f32, tag="ones1")
            bsel = const_pool.tile([128, 4], f32, tag="bsel")

            nc.sync.dma_start(u7[:], u7_d[:])
            nc.sync.dma_start(s2[:], s2_d[:])
            nc.sync.dma_start(bsel[:], bsel_d[:])
            for m in range(nmov):
                nc.sync.dma_start(movs[:, m * 256:(m + 1) * 256], movs_d[m])
            nc.vector.memset(ones1[:], 1.0)

            # ---- init: L1 = u7 (x) v5 (both planes) -----------------------
            # L1 col of (b, l^) plane ri: (b>>2)*256 + ri*128 + (b&3)*32 + l^
            for plane, v5d in ((0, v5r_d), (1, v5i_d)):
                for g in range(16):          # 512 cols = 16 batches each
                    v5c = v5_pool.tile([1, 512], f32, tag="v5c")
                    nc.sync.dma_start(v5c[:], v5d[:, g * 512:(g + 1) * 512])
                    pb = psum_init.tile([128, 512], f32, tag="initb")
                    nc.tensor.matmul(pb[:], ones1[:], v5c[:], start=True, stop=True)
                    out_ap = mk_ap(L1[:, g * 4 * 256 + plane * 128:],
                                   [[256, 4], [32, 4], [1, 32]])
                    in0 = mk_ap(pb[:], [[128, 4], [32, 4], [1, 32]])
                    in1 = mk_ap(u7[:, g * 16:], [[4, 4], [1, 4], [0, 32]])
                    nc.vector.tensor_tensor(out_ap, in0, in1, mybir.AluOpType.mult)

            # probs buffer (fp32) for the fused last-pass readout
            SQ = None
            if dtype_name != "float32":
                SQ = state_pool.tile([128, NBH * 256], f32, tag="SQ")

            # ---- 15 passes ------------------------------------------------
            npass = len(pass_meta)
            for ip, pinfo in enumerate(pass_meta):
                p1 = pinfo['kind'] == 'P1'
                last = (ip == npass - 1) and SQ is not None
                src, dst = (L1, L2) if p1 else (L2, L1)
                for grp in range(NBH // GRP):
                    ps = psum_acc.tile([128, GRP * 256], f32, tag="acc")
                    for ci in range(GRP):
                        ch = grp * GRP + ci
                        base = ch * 256
                        for (tile_col, mcols, ir, ii) in pinfo['cls_idx']:
                            if p1:
                                if mcols == 64:
                                    w7off = 1 if tile_col else 0
                                    stat_r = mk_ap(src[:, base + w7off:],
                                                   [[32, 4], [2, 16]])
                                    stat_i = mk_ap(src[:, base + 128 + w7off:],
                                                   [[32, 4], [2, 16]])
                                else:
                                    stat_r = mk_ap(src[:, base:],
                                                   [[1, 2], [32, 4], [2, 16]])
                                    stat_i = mk_ap(src[:, base + 128:],
                                                   [[1, 2], [32, 4], [2, 16]])
                            else:
                                stat_r = src[:, base + tile_col:base + tile_col + mcols]
                                stat_i = src[:, base + 128 + tile_col:base + 128 + tile_col + mcols]
                            out_ps = ps[tile_col:tile_col + mcols, ci * 256:(ci + 1) * 256]
                            tp = (0, tile_col) if mcols == 64 else None
                            mv_r = movs[:, ir * 256:(ir + 1) * 256]
                            mv_i = movs[:, ii * 256:(ii + 1) * 256]
                            nc.tensor.matmul(out_ps, stat_r, mv_r,
                                             start=True, stop=False, tile_position=tp)
                            nc.tensor.matmul(out_ps, stat_i, mv_i,
                                             start=False, stop=True, tile_position=tp)
                    lo, hi = grp * GRP * 256, (grp + 1) * GRP * 256
                    if last:
                        # fuse eviction with |amp|^2 (ACT square, PSUM src)
                        nc.scalar.activation(SQ[:, lo:hi], ps[:],
                                             mybir.ActivationFunctionType.Square)
                    elif grp % 2 == 0:
                        nc.vector.tensor_copy(dst_ap := dst[:, lo:hi], ps[:])
                    else:
                        nc.scalar.copy(dst[:, lo:hi], ps[:])

            # ---- readout --------------------------------------------------
            if SQ is None:
                SQ = L2                      # fp32 path: square in place
                nc.vector.tensor_tensor(SQ[:], L2[:], L2[:], mybir.AluOpType.mult)
            s2b = mk_ap(s2[:], [[0, NBH], [1, 256]])
            sqseg = mk_ap(SQ[:], [[256, NBH], [1, 256]])
            nc.vector.tensor_tensor(sqseg, sqseg, s2b, mybir.AluOpType.mult)
            R1 = small_pool.tile([128, NBH], f32, tag="R1")
            nc.vector.tensor_reduce(R1[:], sqseg, axis=mybir.AxisListType.X,
                                    op=mybir.AluOpType.add)
            pr = psum_rd.tile([4, NBH], f32, tag="rd")
            nc.tensor.matmul(pr[:], bsel[:], R1[:], start=True, stop=True)
            res = small_pool.tile([4, NBH], f32, tag="res")
            nc.scalar.copy(res[:], pr[:])
            out_ap = bass.AP(out_d.tensor, out_d.offset, [[1, 4], [4, NBH]])
            nc.sync.dma_start(out_ap, res[:])

    nc.compile()
    return nc


_PROGRAM_CACHE = {}


def _prepare_host(q_weights, W_out):
    passes = _build_movings(q_weights)
    mov_blobs, pass_meta = [], []
    for pinfo in passes:
        cls_idx = []
        for (tile_col, mcols, movr, movi) in pinfo['classes']:
            cls_idx.append((tile_col, mcols, len(mov_blobs), len(mov_blobs) + 1))
            mov_blobs.append(movr)
            mov_blobs.append(movi)
        pass_meta.append(dict(kind=pinfo['kind'], cls_idx=cls_idx))
    movs_arr = np.stack(mov_blobs).astype(np.float32)

    s_tile = _sign_tile(W_out).astype(np.float32)       # [h^, l^]
    kk = np.arange(128)
    lk = (kk & 15) * 2 + (kk >> 6)
    s2 = np.zeros((128, 256), dtype=np.float32)
    for k in range(128):
        s2[k, 0:128] = s_tile[:, lk[k]]
        s2[k, 128:256] = s_tile[:, lk[k]]
    return pass_meta, movs_arr, s2


def kernel(inputs, W_in, q_weights, W_out, b_out):
    from concourse.bass_utils import run_bass_kernel_spmd

    inputs = np.asarray(inputs, dtype=np.float32)
    W_in = np.asarray(W_in, dtype=np.float32)
    q_weights = np.asarray(q_weights, dtype=np.float32)
    W_out = np.asarray(W_out, dtype=np.float32)
    b_out = np.asarray(b_out, dtype=np.float32)

    x = inputs.astype(np.float64) @ W_in.T.astype(np.float64)     # [B, 12]
    u7o, v5o = _init_factors(x, q_weights.astype(np.float64))
    pass_meta, movs_arr, s2 = _prepare_host(q_weights, W_out)

    key = (DTYPE_NAME, movs_arr.shape[0])
    if key not in _PROGRAM_CACHE:
        _PROGRAM_CACHE[key] = _build_program(pass_meta, movs_arr.shape[0],
                                             DTYPE_NAME)
    nc = _PROGRAM_CACHE[key]

    movs_cast = movs_arr
    if DTYPE_NAME == "float16":
        movs_cast = movs_arr.astype(np.float16)
    elif DTYPE_NAME == "bfloat16":
        import ml_dtypes
        movs_cast = movs_arr.astype(ml_dtypes.bfloat16)
    kkp = np.arange(128)
    bsel_np = np.zeros((128, 4), dtype=np.float32)
    bsel_np[kkp, (kkp >> 4) & 3] = 1.0
    in_maps = []
    for c in range(NCORES):
        sl = slice(c * BLOC, (c + 1) * BLOC)
        in_maps.append({
            "u7": u7o[sl].T.astype(np.float32).copy(),
            "v5r": v5o[sl].real.astype(np.float32).reshape(1, -1).copy(),
            "v5i": v5o[sl].imag.astype(np.float32).reshape(1, -1).copy(),
            "s2": s2,
            "bsel": bsel_np,
            "movs": movs_cast,
        })
    trace = bool(int(os.environ.get("QKERNEL_TRACE", "0")))
    res = run_bass_kernel_spmd(nc, in_maps, list(range(NCORES)), trace=trace)
    global _LAST_RESULTS
    _LAST_RESULTS = res
    out = np.concatenate([res.results[c]["out"] for c in range(NCORES)])
    return (out + b_out[0]).astype(np.float32)[:, None]


_LAST_RESULTS = None


# revision 13
# speedup vs baseline: 2.6507x; 1.0390x over previous
"""Trainium2 Bass kernel for nn_DQNNModel (12-qubit, 8-layer DQNN, B=2048).

Self-contained: host-side numpy builds all gate/moving matrices; the device
runs a 15-pass state-stationary matmul pipeline over the [256, 4096] complex
statevector per core (batch sharded 8 ways across NeuronCores).

Math design (verified against the jax reference in numpy):
 - wires 0..11, wire w <-> index bit (11-w). G1 = wires 0..6 (128), G2 =
   wires 7..11 (32). Custom orders: G1 bits (w0, w6, w1..w5); G2 bits
   (w11, w8, w9, w10, w7).
 - Per layer j<=6: CNOT(6,7) = H7.CZ67.H7 and CNOT(11,0) = H0.CZ110.H0.
   CZs are diagonal and fold into class-split (col-tiled) matmuls whose
   movings carry sign flips; each layer's G2 gates ride in the PREVIOUS
   G2 pass (after the conditional Z), layer 0's in the initial product
   state. Layer 7's whole CNOT ring folds into the readout sign tile.
 - Passes: P1_j (contract G1) j=0..7, P2_j (contract G2) j=0..6.
   Layouts: L1 [p=G1^(128), f=(b_hi 64, ri 2, b_lo 4, rest1^ 32)],
            L2 [p=(w7, b_lo, q4)(128), f=(b_hi 64, ri 2, G1^ 128)].
"""
import os
import numpy as np

NQ, NL, SEQ, B, DIM = 12, 8, 64, 2048, 4096
NCORES = 8
BLOC = B // NCORES          # 256
NBH = BLOC // 4             # 64 b_hi chunks (4 batches each)
GRP = 4                     # chunks per PSUM group / eviction instr
DTYPE_NAME = os.environ.get("QKERNEL_DTYPE", "float32r")

# ---------------------------------------------------------------- host math
H2C = (1.0 / np.sqrt(2.0)) * np.array([[1, 1], [1, -1]], dtype=np.complex128)
I2C = np.eye(2, dtype=np.complex128)
G1_WIRES = [0, 6, 1, 2, 3, 4, 5]
G2_WIRES = [11, 8, 9, 10, 7]
G1_NAT = [0, 1, 2, 3, 4, 5, 6]
G2_NAT = [7, 8, 9, 10, 11]


def _order_perm(custom_wires, nat_wires):
    n = len(custom_wires)
    perm = np.zeros(2 ** n, dtype=np.int64)
    for ci in range(2 ** n):
        bits = {w: (ci >> (n - 1 - pos)) & 1 for pos, w in enumerate(custom_wires)}
        ni = 0
        for pos, w in enumerate(nat_wires):
            ni |= bits[w] << (n - 1 - pos)
        perm[ci] = ni
    return perm


P1O = _order_perm(G1_WIRES, G1_NAT)
P2O = _order_perm(G2_WIRES, G2_NAT)


def _kron_list(mats):
    out = np.array([[1.0]], dtype=np.complex128)
    for m in mats:
        out = np.kron(out, m)
    return out


def _op_on(group_nat, wire, U):
    return _kron_list([U if w == wire else I2C for w in group_nat])


def _cnot_in(group_nat, c, t):
    n = len(group_nat)
    pos = {w: n - 1 - i for i, w in enumerate(group_nat)}
    dim = 2 ** n
    M = np.zeros((dim, dim), dtype=np.complex128)
    for k in range(dim):
        cb = (k >> pos[c]) & 1
        M[k ^ (cb << pos[t]), k] = 1.0
    return M


def _rot_matrices(qw_layer):
    out = []
    for i in range(NQ):
        a, bb, g = qw_layer[i, 0] * 0.5, qw_layer[i, 1] * 0.5, qw_layer[i, 2] * 0.5
        ca, sa = np.cos(a), np.sin(a)
        cb, sb = np.cos(bb), np.sin(bb)
        m00 = cb * ca + 1j * sb * sa
        m01 = -(sb * ca) - 1j * cb * sa
        m10 = sb * ca - 1j * cb * sa
        m11 = cb * ca - 1j * sb * sa
        ez = np.exp(-1j * g)
        out.append(np.array([[ez * m00, ez * m01],
                             [np.conj(ez) * m10, np.conj(ez) * m11]]))
    return out


def _relabel(M, perm):
    return M[np.ix_(perm, perm)]


def _build_layer_ops(q_weights):
    H0 = _op_on(G1_NAT, 0, H2C)
    H7 = _op_on(G2_NAT, 7, H2C)
    Ch = np.eye(128, dtype=np.complex128)
    for c in range(6):
        Ch = _cnot_in(G1_NAT, c, c + 1) @ Ch
    Cl = np.eye(32, dtype=np.complex128)
    for c in range(7, 11):
        Cl = _cnot_in(G2_NAT, c, c + 1) @ Cl
    A_list, Bpre_list, Bpost_list = [], [], []
    for j in range(NL):
        Us = _rot_matrices(q_weights[:, j, :])
        UG1 = _kron_list([Us[w] for w in G1_NAT])
        conj = (j <= NL - 2)
        A = Ch @ UG1 if conj else UG1
        if conj:
            A = H0 @ A
        if j >= 1:
            A = A @ H0
        A_list.append(_relabel(A, P1O))
        if conj:
            Us_n = _rot_matrices(q_weights[:, j + 1, :])
            UG2n = _kron_list([Us_n[w] for w in G2_NAT])
            Bpre = Cl @ H7
            Bpost = UG2n if j + 1 > NL - 2 else (H7 @ UG2n)
            Bpre_list.append(_relabel(Bpre, P2O))
            Bpost_list.append(_relabel(Bpost, P2O))
    return A_list, Bpre_list, Bpost_list


def _init_factors(x, q_weights):
    ang = (np.pi * 0.5) * x.astype(np.float64)
    c, s = np.cos(ang), np.sin(ang)
    Bsz = x.shape[0]
    u = np.ones((Bsz, 1), dtype=np.float64)
    for w in G1_WIRES:
        u = (u[:, :, None] * np.stack([c[:, w], s[:, w]], -1)[:, None, :]).reshape(Bsz, -1)
    Us0 = _rot_matrices(q_weights[:, 0, :])
    v = np.ones((Bsz, 1), dtype=np.complex128)
    for w in G2_WIRES:
        f = np.stack([c[:, w], s[:, w]], -1).astype(np.complex128)
        M = Us0[w]
        if w == 7:
            M = H2C @ M
        f = f @ M.T
        v = (v[:, :, None] * f[:, None, :]).reshape(Bsz, -1)
    return u, v


def _ring_perm_map():
    F = np.arange(DIM, dtype=np.int64)
    pairs = [(c, c + 1) for c in range(NQ - 1)] + [(NQ - 1, 0)]
    for c, t in pairs:
        pc, pt = NQ - 1 - c, NQ - 1 - t
        v = np.arange(DIM)
        F = F[v ^ (((v >> pc) & 1) << pt)]
    return F


def _sign_tile(W_out):
    k = np.arange(DIM)
    signs = (1.0 - 2.0 * ((k[None, :] >> (NQ - 1 - np.arange(NQ))[:, None]) & 1))
    s_nat = (W_out[0].astype(np.float64) @ signs)
    s_eff = s_nat[np.argsort(_ring_perm_map())]
    kmat = (P1O[:, None] << 5) | P2O[None, :]
    return s_eff[kmat]                                  # [128, 32] (p^, l^)


def _build_movings(q_weights):
    """Per pass: dict(kind, classes=[(tile_col, m_cols, mov_r, mov_i)])."""
    A_list, Bpre_list, Bpost_list = _build_layer_ops(q_weights.astype(np.float64))
    passes = []
    h_idx = np.arange(128)
    negmask_h = ((h_idx & 32) == 32)                    # Z6: negate w6'=1 cols
    l_idx = np.arange(32)
    z11 = np.where((l_idx & 16) == 16, -1.0, 1.0)       # Z11 (w11 = l^ MSB)
    kk = np.arange(128)
    blk, lk = (kk >> 4) & 3, (kk & 15) * 2 + (kk >> 6)  # L2 part -> (b_lo, l^)

    for j in range(NL):
        conj = (j <= NL - 2)
        A = A_list[j]
        Ar, Ai = A.real.astype(np.float32), A.imag.astype(np.float32)
        mov_r = np.concatenate([Ar.T, Ai.T], axis=1).astype(np.float32)
        mov_i = np.concatenate([-Ai.T, Ar.T], axis=1).astype(np.float32)
        if conj:
            neg = np.concatenate([negmask_h, negmask_h])
            mov_r_B = mov_r.copy(); mov_r_B[:, neg] *= -1.0
            mov_i_B = mov_i.copy(); mov_i_B[:, neg] *= -1.0
            classes = [(0, 64, mov_r, mov_i), (64, 64, mov_r_B, mov_i_B)]
        else:
            # split into w7-halves anyway (walrus: weights AP must coalesce
            # to one free dim); same movings for both halves
            classes = [(0, 64, mov_r, mov_i), (64, 64, mov_r, mov_i)]
        passes.append(dict(kind='P1', classes=classes))
        if conj:
            MA = Bpost_list[j] @ Bpre_list[j]
            MB = Bpost_list[j] @ (z11[:, None] * Bpre_list[j])
            cls = []
            for tile_col, Mc in ((0, MA), (64, MB)):
                Mr = Mc.real.astype(np.float32)
                Mi = Mc.imag.astype(np.float32)
                movr = np.zeros((128, 256), dtype=np.float32)
                movi = np.zeros((128, 256), dtype=np.float32)
                for k in range(128):
                    base = blk[k] * 32
                    movr[k, base:base + 32] = Mr[:, lk[k]]
                    movr[k, 128 + base:128 + base + 32] = Mi[:, lk[k]]
                    movi[k, base:base + 32] = -Mi[:, lk[k]]
                    movi[k, 128 + base:128 + base + 32] = Mr[:, lk[k]]
                cls.append((tile_col, 64, movr, movi))
            passes.append(dict(kind='P2', classes=cls))
    return passes


# ---------------------------------------------------------------- device
def _build_program(pass_meta, nmov, dtype_name):
    """pass_meta: list of dict(kind, cls_idx=[(tile_col, m_cols, ir, ii)])."""
    import concourse.bass as bass
    import concourse.tile as tile
    from concourse import bacc, mybir

    f32 = mybir.dt.float32
    mmdt = getattr(mybir.dt, dtype_name)

    nc = bacc.Bacc("TRN2", target_bir_lowering=False, debug=False)
    u7_d = nc.dram_tensor("u7", [128, BLOC], f32, kind="ExternalInput").ap()
    v5r_d = nc.dram_tensor("v5r", [1, BLOC * 32], f32, kind="ExternalInput").ap()
    v5i_d = nc.dram_tensor("v5i", [1, BLOC * 32], f32, kind="ExternalInput").ap()
    s2_d = nc.dram_tensor("s2", [128, 256], f32, kind="ExternalInput").ap()
    bsel_d = nc.dram_tensor("bsel", [128, 4], f32, kind="ExternalInput").ap()
    movs_d = nc.dram_tensor("movs", [nmov, 128, 256], mmdt, kind="ExternalInput").ap()
    out_d = nc.dram_tensor("out", [BLOC], f32, kind="ExternalOutput").ap()

    def mk_ap(base_ap, dims):
        return bass.AP(base_ap.tensor, base_ap.offset, [list(base_ap.ap[0])] + dims)

    with tile.TileContext(nc) as tc:
        with (
            tc.tile_pool(name="state", bufs=1) as state_pool,
            tc.tile_pool(name="const", bufs=1) as const_pool,
            tc.tile_pool(name="pinit", bufs=1, space="PSUM") as psum_init,
            tc.tile_pool(name="pacc", bufs=3, space="PSUM") as psum_acc,
            tc.tile_pool(name="prd", bufs=1, space="PSUM") as psum_rd,
            tc.tile_pool(name="small", bufs=1) as small_pool,
        ):
            L1 = state_pool.tile([128, NBH * 256], mmdt, tag="L1")
            L2 = state_pool.tile([128, NBH * 256], mmdt, tag="L2")
            movs = const_pool.tile([128, nmov * 256], mmdt, tag="movs")
            u7 = const_pool.tile([128, BLOC], f32, tag="u7")
            s2 = const_pool.tile([128, 256], f32, tag="s2")
            ones1 = const_pool.tile([1, 128], f32, tag="ones1")
            bsel = const_pool.tile([128, 4], f32, tag="bsel")

            nc.sync.dma_start(u7[:], u7_d[:])
            nc.sync.dma_start(s2[:], s2_d[:])
            nc.sync.dma_start(bsel[:], bsel_d[:])
            for m in range(nmov):
                nc.sync.dma_start(movs[:, m * 256:(m + 1) * 256], movs_d[m])
            nc.vector.memset(ones1[:], 1.0)
            R1 = small_pool.tile([128, NBH], f32, tag="R1")
            # probs buffer (fp32) for the fused last-pass readout; its first
            # row also hosts the v5 staging data during init (disjoint in time)
            SQ = None
            if dtype_name != "float32":
                SQ = state_pool.tile([128, NBH * 256], f32, tag="SQ")

            # ---- init: L1 = u7 (x) v5 (both planes) -----------------------
            # L1 col of (b, l^) plane ri: (b>>2)*256 + ri*128 + (b&3)*32 + l^
            assert SQ is not None, "fp32 path needs small-chunk v5 staging"
            v5t = {0: SQ[0:1, 0:BLOC * 32], 1: SQ[0:1, BLOC * 32:2 * BLOC * 32]}
            for plane, v5d in ((0, v5r_d), (1, v5i_d)):
                for q in range(4):
                    nc.sync.dma_start(v5t[plane][:, q * 2048:(q + 1) * 2048],
                                      v5d[:, q * 2048:(q + 1) * 2048])
            for g in range(16):              # 512 cols = 16 batches each
                for plane in (0, 1):
                    pb = psum_init.tile([128, 512], f32, tag="initb")
                    nc.tensor.matmul(pb[:], ones1[:],
                                     v5t[plane][:, g * 512:(g + 1) * 512],
                                     start=True, stop=True)
                    out_ap = mk_ap(L1[:, g * 4 * 256 + plane * 128:],
                                   [[256, 4], [32, 4], [1, 32]])
                    in0 = mk_ap(pb[:], [[128, 4], [32, 4], [1, 32]])
                    in1 = mk_ap(u7[:, g * 16:], [[4, 4], [1, 4], [0, 32]])
                    nc.vector.tensor_tensor(out_ap, in0, in1, mybir.AluOpType.mult)

            # ---- 15 passes ------------------------------------------------
            npass = len(pass_meta)
            for ip, pinfo in enumerate(pass_meta):
                p1 = pinfo['kind'] == 'P1'
                last = (ip == npass - 1) and SQ is not None
                src, dst = (L1, L2) if p1 else (L2, L1)
                for grp in range(NBH // GRP):
                    ps = psum_acc.tile([128, GRP * 256], f32, tag="acc")
                    for ci in range(GRP):
                        ch = grp * GRP + ci
                        base = ch * 256
                        mm_q = []
                        for (tile_col, mcols, ir, ii) in pinfo['cls_idx']:
                            if p1:
                                if mcols == 64:
                                    w7off = 1 if tile_col else 0
                                    stat_r = mk_ap(src[:, base + w7off:],
                                                   [[32, 4], [2, 16]])
                                    stat_i = mk_ap(src[:, base + 128 + w7off:],
                                                   [[32, 4], [2, 16]])
                                else:
                                    stat_r = mk_ap(src[:, base:],
                                                   [[1, 2], [32, 4], [2, 16]])
                                    stat_i = mk_ap(src[:, base + 128:],
                                                   [[1, 2], [32, 4], [2, 16]])
                            else:
                                stat_r = src[:, base + tile_col:base + tile_col + mcols]
                                stat_i = src[:, base + 128 + tile_col:base + 128 + tile_col + mcols]
                            out_ps = ps[tile_col:tile_col + mcols, ci * 256:(ci + 1) * 256]
                            tp = (0, tile_col) if mcols == 64 else None
                            mv_r = movs[:, ir * 256:(ir + 1) * 256]
                            mv_i = movs[:, ii * 256:(ii + 1) * 256]
                            mm_q.append((out_ps, stat_r, mv_r, True, False, tp))
                            mm_q.append((out_ps, stat_i, mv_i, False, True, tp))
                        # emit class-A/B pairs adjacently per plane so the
                        # col-tiled pair runs concurrently in the PE
                        order = [0, 2, 1, 3] if len(mm_q) == 4 else range(len(mm_q))
                        for qi in order:
                            o, st, mv, b0, b1, tp = mm_q[qi]
                            nc.tensor.matmul(o, st, mv, start=b0, stop=b1,
                                             tile_position=tp)
                    lo, hi = grp * GRP * 256, (grp + 1) * GRP * 256
                    if last:
                        # fuse eviction with |amp|^2 (ACT square, PSUM src)
                        nc.scalar.activation(SQ[:, lo:hi], ps[:],
                                             mybir.ActivationFunctionType.Square)
                        sqseg = mk_ap(SQ[:, lo:], [[256, GRP], [1, 256]])
                        s2g = mk_ap(s2[:], [[0, GRP], [1, 256]])
                        nc.vector.tensor_tensor(sqseg, sqseg, s2g,
                                                mybir.AluOpType.mult)
                        nc.vector.tensor_reduce(
                            R1[:, grp * GRP:(grp + 1) * GRP], sqseg,
                            axis=mybir.AxisListType.X, op=mybir.AluOpType.add)
                    elif grp % 2 == 0:
                        nc.vector.tensor_copy(dst_ap := dst[:, lo:hi], ps[:])
                    else:
                        nc.scalar.copy(dst[:, lo:hi], ps[:])

            # ---- readout tail ---------------------------------------------
            if SQ is None:
                SQ = L2                      # fp32 path: square in place
                nc.vector.tensor_tensor(SQ[:], L2[:], L2[:], mybir.AluOpType.mult)
                s2b = mk_ap(s2[:], [[0, NBH], [1, 256]])
                sqseg = mk_ap(SQ[:], [[256, NBH], [1, 256]])
                nc.vector.tensor_tensor(sqseg, sqseg, s2b, mybir.AluOpType.mult)
                nc.vector.tensor_reduce(R1[:], sqseg, axis=mybir.AxisListType.X,
                                        op=mybir.AluOpType.add)
            pr = psum_rd.tile([4, NBH], f32, tag="rd")
            nc.tensor.matmul(pr[:], bsel[:], R1[:], start=True, stop=True)
            res = small_pool.tile([4, NBH], f32, tag="res")
            nc.scalar.copy(res[:], pr[:])
            out_ap = bass.AP(out_d.tensor, out_d.offset, [[1, 4], [4, NBH]])
            nc.sync.dma_start(out_ap, res[:])

    nc.compile()
    return nc


_PROGRAM_CACHE = {}


def _prepare_host(q_weights, W_out):
    passes = _build_movings(q_weights)
    mov_blobs, pass_meta = [], []
    for pinfo in passes:
        cls_idx = []
        for (tile_col, mcols, movr, movi) in pinfo['classes']:
            cls_idx.append((tile_col, mcols, len(mov_blobs), len(mov_blobs) + 1))
            mov_blobs.append(movr)
            mov_blobs.append(movi)
        pass_meta.append(dict(kind=pinfo['kind'], cls_idx=cls_idx))
    movs_arr = np.stack(mov_blobs).astype(np.float32)

    s_tile = _sign_tile(W_out).astype(np.float32)       # [h^, l^]
    kk = np.arange(128)
    lk = (kk & 15) * 2 + (kk >> 6)
    s2 = np.zeros((128, 256), dtype=np.float32)
    for k in range(128):
        s2[k, 0:128] = s_tile[:, lk[k]]
        s2[k, 128:256] = s_tile[:, lk[k]]
    return pass_meta, movs_arr, s2


def kernel(inputs, W_in, q_weights, W_out, b_out):
    from concourse.bass_utils import run_bass_kernel_spmd

    inputs = np.asarray(inputs, dtype=np.float32)
    W_in = np.asarray(W_in, dtype=np.float32)
    q_weights = np.asarray(q_weights, dtype=np.float32)
    W_out = np.asarray(W_out, dtype=np.float32)
    b_out = np.asarray(b_out, dtype=np.float32)

    x = inputs.astype(np.float64) @ W_in.T.astype(np.float64)     # [B, 12]
    u7o, v5o = _init_factors(x, q_weights.astype(np.float64))
    pass_meta, movs_arr, s2 = _prepare_host(q_weights, W_out)

    key = (DTYPE_NAME, movs_arr.shape[0])
    if key not in _PROGRAM_CACHE:
        _PROGRAM_CACHE[key] = _build_program(pass_meta, movs_arr.shape[0],
                                             DTYPE_NAME)
    nc = _PROGRAM_CACHE[key]

    movs_cast = movs_arr
    if DTYPE_NAME == "float16":
        movs_cast = movs_arr.astype(np.float16)
    elif DTYPE_NAME == "bfloat16":
        import ml_dtypes
        movs_cast = movs_arr.astype(ml_dtypes.bfloat16)
    kkp = np.arange(128)
    bsel_np = np.zeros((128, 4), dtype=np.float32)
    bsel_np[kkp, (kkp >> 4) & 3] = 1.0
    in_maps = []
    for c in range(NCORES):
        sl = slice(c * BLOC, (c + 1) * BLOC)
        in_maps.append({
            "u7": u7o[sl].T.astype(np.float32).copy(),
            "v5r": v5o[sl].real.astype(np.float32).reshape(1, -1).copy(),
            "v5i": v5o[sl].imag.astype(np.float32).reshape(1, -1).copy(),
            "s2": s2,
            "bsel": bsel_np,
            "movs": movs_cast,
        })
    trace = bool(int(os.environ.get("QKERNEL_TRACE", "0")))
    res = run_bass_kernel_spmd(nc, in_maps, list(range(NCORES)), trace=trace)
    global _LAST_RESULTS
    _LAST_RESULTS = res
    out = np.concatenate([res.results[c]["out"] for c in range(NCORES)])
    return (out + b_out[0]).astype(np.float32)[:, None]


_LAST_RESULTS = None


# revision 14
# speedup vs baseline: 2.9308x; 1.1057x over previous
"""Trainium2 Bass kernel for nn_DQNNModel (12-qubit, 8-layer DQNN, B=2048).

Self-contained: host-side numpy builds all gate/moving matrices; the device
runs a 15-pass state-stationary matmul pipeline over the [256, 4096] complex
statevector per core (batch sharded 8 ways across NeuronCores).

Math design (verified against the jax reference in numpy):
 - wires 0..11, wire w <-> index bit (11-w). G1 = wires 0..6 (128), G2 =
   wires 7..11 (32). Custom orders: G1 bits (w0, w6, w1..w5); G2 bits
   (w11, w8, w9, w10, w7).
 - Per layer j<=6: CNOT(6,7) = H7.CZ67.H7 and CNOT(11,0) = H0.CZ110.H0.
   CZs are diagonal and fold into class-split (col-tiled) matmuls whose
   movings carry sign flips; each layer's G2 gates ride in the PREVIOUS
   G2 pass (after the conditional Z), layer 0's in the initial product
   state. Layer 7's whole CNOT ring folds into the readout sign tile.
 - Passes: P1_j (contract G1) j=0..7, P2_j (contract G2) j=0..6.
   Layouts: L1 [p=G1^(128), f=(b_hi 64, ri 2, b_lo 4, rest1^ 32)],
            L2 [p=(w7, b_lo, q4)(128), f=(b_hi 64, ri 2, G1^ 128)].
"""
import os
import numpy as np

NQ, NL, SEQ, B, DIM = 12, 8, 64, 2048, 4096
NCORES = 8
BLOC = B // NCORES          # 256
NBH = BLOC // 4             # 64 b_hi chunks (4 batches each)
GRP = 4                     # chunks per PSUM group / eviction instr
DTYPE_NAME = os.environ.get("QKERNEL_DTYPE", "float32r")

# ---------------------------------------------------------------- host math
H2C = (1.0 / np.sqrt(2.0)) * np.array([[1, 1], [1, -1]], dtype=np.complex128)
I2C = np.eye(2, dtype=np.complex128)
G1_WIRES = [0, 6, 1, 2, 3, 4, 5]
G2_WIRES = [11, 8, 9, 10, 7]
G1_NAT = [0, 1, 2, 3, 4, 5, 6]
G2_NAT = [7, 8, 9, 10, 11]


def _order_perm(custom_wires, nat_wires):
    n = len(custom_wires)
    perm = np.zeros(2 ** n, dtype=np.int64)
    for ci in range(2 ** n):
        bits = {w: (ci >> (n - 1 - pos)) & 1 for pos, w in enumerate(custom_wires)}
        ni = 0
        for pos, w in enumerate(nat_wires):
            ni |= bits[w] << (n - 1 - pos)
        perm[ci] = ni
    return perm


P1O = _order_perm(G1_WIRES, G1_NAT)
P2O = _order_perm(G2_WIRES, G2_NAT)


def _kron_list(mats):
    out = np.array([[1.0]], dtype=np.complex128)
    for m in mats:
        out = np.kron(out, m)
    return out


def _op_on(group_nat, wire, U):
    return _kron_list([U if w == wire else I2C for w in group_nat])


def _cnot_in(group_nat, c, t):
    n = len(group_nat)
    pos = {w: n - 1 - i for i, w in enumerate(group_nat)}
    dim = 2 ** n
    M = np.zeros((dim, dim), dtype=np.complex128)
    for k in range(dim):
        cb = (k >> pos[c]) & 1
        M[k ^ (cb << pos[t]), k] = 1.0
    return M


def _rot_matrices(qw_layer):
    out = []
    for i in range(NQ):
        a, bb, g = qw_layer[i, 0] * 0.5, qw_layer[i, 1] * 0.5, qw_layer[i, 2] * 0.5
        ca, sa = np.cos(a), np.sin(a)
        cb, sb = np.cos(bb), np.sin(bb)
        m00 = cb * ca + 1j * sb * sa
        m01 = -(sb * ca) - 1j * cb * sa
        m10 = sb * ca - 1j * cb * sa
        m11 = cb * ca - 1j * sb * sa
        ez = np.exp(-1j * g)
        out.append(np.array([[ez * m00, ez * m01],
                             [np.conj(ez) * m10, np.conj(ez) * m11]]))
    return out


def _relabel(M, perm):
    return M[np.ix_(perm, perm)]


def _build_layer_ops(q_weights):
    H0 = _op_on(G1_NAT, 0, H2C)
    H7 = _op_on(G2_NAT, 7, H2C)
    Ch = np.eye(128, dtype=np.complex128)
    for c in range(6):
        Ch = _cnot_in(G1_NAT, c, c + 1) @ Ch
    Cl = np.eye(32, dtype=np.complex128)
    for c in range(7, 11):
        Cl = _cnot_in(G2_NAT, c, c + 1) @ Cl
    A_list, Bpre_list, Bpost_list = [], [], []
    for j in range(NL):
        Us = _rot_matrices(q_weights[:, j, :])
        UG1 = _kron_list([Us[w] for w in G1_NAT])
        conj = (j <= NL - 2)
        A = Ch @ UG1 if conj else UG1
        if conj:
            A = H0 @ A
        if j >= 1:
            A = A @ H0
        A_list.append(_relabel(A, P1O))
        if conj:
            Us_n = _rot_matrices(q_weights[:, j + 1, :])
            UG2n = _kron_list([Us_n[w] for w in G2_NAT])
            Bpre = Cl @ H7
            Bpost = UG2n if j + 1 > NL - 2 else (H7 @ UG2n)
            Bpre_list.append(_relabel(Bpre, P2O))
            Bpost_list.append(_relabel(Bpost, P2O))
    return A_list, Bpre_list, Bpost_list


def _init_factors(x, q_weights):
    ang = (np.pi * 0.5) * x.astype(np.float64)
    c, s = np.cos(ang), np.sin(ang)
    Bsz = x.shape[0]
    u = np.ones((Bsz, 1), dtype=np.float64)
    for w in G1_WIRES:
        u = (u[:, :, None] * np.stack([c[:, w], s[:, w]], -1)[:, None, :]).reshape(Bsz, -1)
    Us0 = _rot_matrices(q_weights[:, 0, :])
    v = np.ones((Bsz, 1), dtype=np.complex128)
    for w in G2_WIRES:
        f = np.stack([c[:, w], s[:, w]], -1).astype(np.complex128)
        M = Us0[w]
        if w == 7:
            M = H2C @ M
        f = f @ M.T
        v = (v[:, :, None] * f[:, None, :]).reshape(Bsz, -1)
    return u, v


def _ring_perm_map():
    F = np.arange(DIM, dtype=np.int64)
    pairs = [(c, c + 1) for c in range(NQ - 1)] + [(NQ - 1, 0)]
    for c, t in pairs:
        pc, pt = NQ - 1 - c, NQ - 1 - t
        v = np.arange(DIM)
        F = F[v ^ (((v >> pc) & 1) << pt)]
    return F


def _sign_tile(W_out):
    k = np.arange(DIM)
    signs = (1.0 - 2.0 * ((k[None, :] >> (NQ - 1 - np.arange(NQ))[:, None]) & 1))
    s_nat = (W_out[0].astype(np.float64) @ signs)
    s_eff = s_nat[np.argsort(_ring_perm_map())]
    kmat = (P1O[:, None] << 5) | P2O[None, :]
    return s_eff[kmat]                                  # [128, 32] (p^, l^)


def _build_movings(q_weights):
    """Per pass: dict(kind, classes=[(tile_col, m_cols, mov_r, mov_i)])."""
    A_list, Bpre_list, Bpost_list = _build_layer_ops(q_weights.astype(np.float64))
    passes = []
    h_idx = np.arange(128)
    negmask_h = ((h_idx & 32) == 32)                    # Z6: negate w6'=1 cols
    l_idx = np.arange(32)
    z11 = np.where((l_idx & 16) == 16, -1.0, 1.0)       # Z11 (w11 = l^ MSB)
    kk = np.arange(128)
    blk, lk = (kk >> 4) & 3, (kk & 15) * 2 + (kk >> 6)  # L2 part -> (b_lo, l^)

    for j in range(NL):
        conj = (j <= NL - 2)
        A = A_list[j]
        Ar, Ai = A.real.astype(np.float32), A.imag.astype(np.float32)
        mov_r = np.concatenate([Ar.T, Ai.T], axis=1).astype(np.float32)
        mov_i = np.concatenate([-Ai.T, Ar.T], axis=1).astype(np.float32)
        if conj:
            neg = np.concatenate([negmask_h, negmask_h])
            mov_r_B = mov_r.copy(); mov_r_B[:, neg] *= -1.0
            mov_i_B = mov_i.copy(); mov_i_B[:, neg] *= -1.0
            classes = [(0, 64, mov_r, mov_i), (64, 64, mov_r_B, mov_i_B)]
        else:
            # split into w7-halves anyway (walrus: weights AP must coalesce
            # to one free dim); same movings for both halves
            classes = [(0, 64, mov_r, mov_i), (64, 64, mov_r, mov_i)]
        passes.append(dict(kind='P1', classes=classes))
        if conj:
            MA = Bpost_list[j] @ Bpre_list[j]
            MB = Bpost_list[j] @ (z11[:, None] * Bpre_list[j])
            cls = []
            for tile_col, Mc in ((0, MA), (64, MB)):
                Mr = Mc.real.astype(np.float32)
                Mi = Mc.imag.astype(np.float32)
                movr = np.zeros((128, 256), dtype=np.float32)
                movi = np.zeros((128, 256), dtype=np.float32)
                for k in range(128):
                    base = blk[k] * 32
                    movr[k, base:base + 32] = Mr[:, lk[k]]
                    movr[k, 128 + base:128 + base + 32] = Mi[:, lk[k]]
                    movi[k, base:base + 32] = -Mi[:, lk[k]]
                    movi[k, 128 + base:128 + base + 32] = Mr[:, lk[k]]
                cls.append((tile_col, 64, movr, movi))
            passes.append(dict(kind='P2', classes=cls))
    return passes


# ---------------------------------------------------------------- device
def _build_program(pass_meta, nmov, dtype_name):
    """pass_meta: list of dict(kind, cls_idx=[(tile_col, m_cols, ir, ii)])."""
    import concourse.bass as bass
    import concourse.tile as tile
    from concourse import bacc, mybir

    f32 = mybir.dt.float32
    mmdt = getattr(mybir.dt, dtype_name)

    nc = bacc.Bacc("TRN2", target_bir_lowering=False, debug=False)
    u7_d = nc.dram_tensor("u7", [128, BLOC], f32, kind="ExternalInput").ap()
    v5r_d = nc.dram_tensor("v5r", [1, BLOC * 32], f32, kind="ExternalInput").ap()
    v5i_d = nc.dram_tensor("v5i", [1, BLOC * 32], f32, kind="ExternalInput").ap()
    s2_d = nc.dram_tensor("s2", [128, 256], f32, kind="ExternalInput").ap()
    bsel_d = nc.dram_tensor("bsel", [128, 4], f32, kind="ExternalInput").ap()
    movs_d = nc.dram_tensor("movs", [nmov, 128, 256], mmdt, kind="ExternalInput").ap()
    out_d = nc.dram_tensor("out", [BLOC], f32, kind="ExternalOutput").ap()

    def mk_ap(base_ap, dims):
        return bass.AP(base_ap.tensor, base_ap.offset, [list(base_ap.ap[0])] + dims)

    with tile.TileContext(nc) as tc:
        with (
            tc.tile_pool(name="state", bufs=1) as state_pool,
            tc.tile_pool(name="const", bufs=1) as const_pool,
            tc.tile_pool(name="pinit", bufs=1, space="PSUM") as psum_init,
            tc.tile_pool(name="pacc", bufs=3, space="PSUM") as psum_acc,
            tc.tile_pool(name="prd", bufs=1, space="PSUM") as psum_rd,
            tc.tile_pool(name="small", bufs=1) as small_pool,
        ):
            L1 = state_pool.tile([128, NBH * 256], mmdt, tag="L1")
            L2 = state_pool.tile([128, NBH * 256], mmdt, tag="L2")
            movs = const_pool.tile([128, nmov * 256], mmdt, tag="movs")
            u7 = const_pool.tile([128, BLOC], f32, tag="u7")
            s2 = const_pool.tile([128, 256], f32, tag="s2")
            ones1 = const_pool.tile([1, 128], f32, tag="ones1")
            bsel = const_pool.tile([128, 4], f32, tag="bsel")

            nc.sync.dma_start(u7[:], u7_d[:])
            nc.sync.dma_start(s2[:], s2_d[:])
            nc.sync.dma_start(bsel[:], bsel_d[:])
            nc.vector.memset(ones1[:], 1.0)
            R1 = small_pool.tile([128, NBH], f32, tag="R1")
            # probs buffer (fp32) for the fused last-pass readout; its first
            # row also hosts the v5 staging data during init (disjoint in time)
            SQ = None
            if dtype_name != "float32":
                SQ = state_pool.tile([128, NBH * 256], f32, tag="SQ")

            # ---- init: L1 = u7 (x) v5 (both planes) -----------------------
            # L1 col of (b, l^) plane ri: (b>>2)*256 + ri*128 + (b&3)*32 + l^
            assert SQ is not None, "fp32 path needs small-chunk v5 staging"
            v5t = {0: SQ[0:1, 0:BLOC * 32], 1: SQ[0:1, BLOC * 32:2 * BLOC * 32]}
            for plane, v5d in ((0, v5r_d), (1, v5i_d)):
                for q in range(4):
                    nc.sync.dma_start(v5t[plane][:, q * 2048:(q + 1) * 2048],
                                      v5d[:, q * 2048:(q + 1) * 2048])
            # movs: one large DMA (emitted after init-critical transfers)
            movs_out = mk_ap(movs[:], [[256, nmov], [1, 256]])
            movs_in = bass.AP(movs_d.tensor, movs_d.offset,
                              [[256, 128], [128 * 256, nmov], [1, 256]])
            nc.sync.dma_start(movs_out, movs_in)
            for g in range(16):              # 512 cols = 16 batches each
                for plane in (0, 1):
                    pb = psum_init.tile([128, 512], f32, tag="initb")
                    nc.tensor.matmul(pb[:], ones1[:],
                                     v5t[plane][:, g * 512:(g + 1) * 512],
                                     start=True, stop=True)
                    out_ap = mk_ap(L1[:, g * 4 * 256 + plane * 128:],
                                   [[256, 4], [32, 4], [1, 32]])
                    in0 = mk_ap(pb[:], [[128, 4], [32, 4], [1, 32]])
                    in1 = mk_ap(u7[:, g * 16:], [[4, 4], [1, 4], [0, 32]])
                    nc.vector.tensor_tensor(out_ap, in0, in1, mybir.AluOpType.mult)

            # ---- 15 passes ------------------------------------------------
            npass = len(pass_meta)
            for ip, pinfo in enumerate(pass_meta):
                p1 = pinfo['kind'] == 'P1'
                last = (ip == npass - 1) and SQ is not None
                src, dst = (L1, L2) if p1 else (L2, L1)
                for grp in range(NBH // GRP):
                    ps = psum_acc.tile([128, GRP * 256], f32, tag="acc")
                    for ci in range(GRP):
                        ch = grp * GRP + ci
                        base = ch * 256
                        mm_q = []
                        for (tile_col, mcols, ir, ii) in pinfo['cls_idx']:
                            if p1:
                                if mcols == 64:
                                    w7off = 1 if tile_col else 0
                                    stat_r = mk_ap(src[:, base + w7off:],
                                                   [[32, 4], [2, 16]])
                                    stat_i = mk_ap(src[:, base + 128 + w7off:],
                                                   [[32, 4], [2, 16]])
                                else:
                                    stat_r = mk_ap(src[:, base:],
                                                   [[1, 2], [32, 4], [2, 16]])
                                    stat_i = mk_ap(src[:, base + 128:],
                                                   [[1, 2], [32, 4], [2, 16]])
                            else:
                                stat_r = src[:, base + tile_col:base + tile_col + mcols]
                                stat_i = src[:, base + 128 + tile_col:base + 128 + tile_col + mcols]
                            out_ps = ps[tile_col:tile_col + mcols, ci * 256:(ci + 1) * 256]
                            tp = (0, tile_col) if mcols == 64 else None
                            mv_r = movs[:, ir * 256:(ir + 1) * 256]
                            mv_i = movs[:, ii * 256:(ii + 1) * 256]
                            mm_q.append((out_ps, stat_r, mv_r, True, False, tp))
                            mm_q.append((out_ps, stat_i, mv_i, False, True, tp))
                        # emit class-A/B pairs adjacently per plane so the
                        # col-tiled pair runs concurrently in the PE
                        order = [0, 2, 1, 3] if len(mm_q) == 4 else range(len(mm_q))
                        for qi in order:
                            o, st, mv, b0, b1, tp = mm_q[qi]
                            nc.tensor.matmul(o, st, mv, start=b0, stop=b1,
                                             tile_position=tp)
                    lo, hi = grp * GRP * 256, (grp + 1) * GRP * 256
                    if last:
                        # fuse eviction with |amp|^2 (ACT square, PSUM src)
                        nc.scalar.activation(SQ[:, lo:hi], ps[:],
                                             mybir.ActivationFunctionType.Square)
                        sqseg = mk_ap(SQ[:, lo:], [[256, GRP], [1, 256]])
                        s2g = mk_ap(s2[:], [[0, GRP], [1, 256]])
                        nc.gpsimd.tensor_tensor(sqseg, sqseg, s2g,
                                                mybir.AluOpType.mult)
                        nc.vector.tensor_reduce(
                            R1[:, grp * GRP:(grp + 1) * GRP], sqseg,
                            axis=mybir.AxisListType.X, op=mybir.AluOpType.add)
                    elif grp % 2 == 0:
                        nc.vector.tensor_copy(dst_ap := dst[:, lo:hi], ps[:])
                    else:
                        nc.scalar.copy(dst[:, lo:hi], ps[:])

            # ---- readout tail ---------------------------------------------
            if SQ is None:
                SQ = L2                      # fp32 path: square in place
                nc.vector.tensor_tensor(SQ[:], L2[:], L2[:], mybir.AluOpType.mult)
                s2b = mk_ap(s2[:], [[0, NBH], [1, 256]])
                sqseg = mk_ap(SQ[:], [[256, NBH], [1, 256]])
                nc.vector.tensor_tensor(sqseg, sqseg, s2b, mybir.AluOpType.mult)
                nc.vector.tensor_reduce(R1[:], sqseg, axis=mybir.AxisListType.X,
                                        op=mybir.AluOpType.add)
            pr = psum_rd.tile([4, NBH], f32, tag="rd")
            nc.tensor.matmul(pr[:], bsel[:], R1[:], start=True, stop=True)
            res = small_pool.tile([4, NBH], f32, tag="res")
            nc.scalar.copy(res[:], pr[:])
            out_ap = bass.AP(out_d.tensor, out_d.offset, [[1, 4], [4, NBH]])
            nc.sync.dma_start(out_ap, res[:])

    nc.compile()
    return nc


_PROGRAM_CACHE = {}


def _prepare_host(q_weights, W_out):
    passes = _build_movings(q_weights)
    mov_blobs, pass_meta = [], []
    for pinfo in passes:
        cls_idx = []
        for (tile_col, mcols, movr, movi) in pinfo['classes']:
            cls_idx.append((tile_col, mcols, len(mov_blobs), len(mov_blobs) + 1))
            mov_blobs.append(movr)
            mov_blobs.append(movi)
        pass_meta.append(dict(kind=pinfo['kind'], cls_idx=cls_idx))
    movs_arr = np.stack(mov_blobs).astype(np.float32)

    s_tile = _sign_tile(W_out).astype(np.float32)       # [h^, l^]
    kk = np.arange(128)
    lk = (kk & 15) * 2 + (kk >> 6)
    s2 = np.zeros((128, 256), dtype=np.float32)
    for k in range(128):
        s2[k, 0:128] = s_tile[:, lk[k]]
        s2[k, 128:256] = s_tile[:, lk[k]]
    return pass_meta, movs_arr, s2


def kernel(inputs, W_in, q_weights, W_out, b_out):
    from concourse.bass_utils import run_bass_kernel_spmd

    inputs = np.asarray(inputs, dtype=np.float32)
    W_in = np.asarray(W_in, dtype=np.float32)
    q_weights = np.asarray(q_weights, dtype=np.float32)
    W_out = np.asarray(W_out, dtype=np.float32)
    b_out = np.asarray(b_out, dtype=np.float32)

    x = inputs.astype(np.float64) @ W_in.T.astype(np.float64)     # [B, 12]
    u7o, v5o = _init_factors(x, q_weights.astype(np.float64))
    pass_meta, movs_arr, s2 = _prepare_host(q_weights, W_out)

    key = (DTYPE_NAME, movs_arr.shape[0])
    if key not in _PROGRAM_CACHE:
        _PROGRAM_CACHE[key] = _build_program(pass_meta, movs_arr.shape[0],
                                             DTYPE_NAME)
    nc = _PROGRAM_CACHE[key]

    movs_cast = movs_arr
    if DTYPE_NAME == "float16":
        movs_cast = movs_arr.astype(np.float16)
    elif DTYPE_NAME == "bfloat16":
        import ml_dtypes
        movs_cast = movs_arr.astype(ml_dtypes.bfloat16)
    kkp = np.arange(128)
    bsel_np = np.zeros((128, 4), dtype=np.float32)
    bsel_np[kkp, (kkp >> 4) & 3] = 1.0
    in_maps = []
    for c in range(NCORES):
        sl = slice(c * BLOC, (c + 1) * BLOC)
        in_maps.append({
            "u7": u7o[sl].T.astype(np.float32).copy(),
            "v5r": v5o[sl].real.astype(np.float32).reshape(1, -1).copy(),
            "v5i": v5o[sl].imag.astype(np.float32).reshape(1, -1).copy(),
            "s2": s2,
            "bsel": bsel_np,
            "movs": movs_cast,
        })
    trace = bool(int(os.environ.get("QKERNEL_TRACE", "0")))
    res = run_bass_kernel_spmd(nc, in_maps, list(range(NCORES)), trace=trace)
    global _LAST_RESULTS
    _LAST_RESULTS = res
    out = np.concatenate([res.results[c]["out"] for c in range(NCORES)])
    return (out + b_out[0]).astype(np.float32)[:, None]


_LAST_RESULTS = None
